# revision 1
# baseline (speedup 1.0000x reference)
"""BotRGCN on 8 Trainium2 NeuronCores (Bass/Tile).

Strategy (1-D destination-sharded graph partition):
  - Host assigns nodes to 8*BPC blocks of 128 destinations via LPT balancing on
    in-degree, so every block holds ~E/(8*BPC) edges; shard s = blocks
    [s*BPC, (s+1)*BPC).  Edges are bucketed per (core, dst-block) and padded to
    a uniform T_pad tiles of 128 edges.
  - Each core: encoder (feature-major matmuls, fp32r, fused Prelu) ->
    node-major x via PE transposes -> AllGather into a replicated table ->
    per dst-block: per-tile indirect-DMA row gather + DVE weighted one-hot
    (tensor_scalar is_equal*mult) + scatter-matmul accumulating
    relation-split sums in PSUM -> per 2-block unit: W_rel / W_root transform
    matmuls + bias -> layer output (feature-major) and next gather table.
  - Layer 2 identical; head = two matmuls + Prelu; per-core output [2, SHARD],
    host inverse-permutes to [N, 2].
"""

import numpy as np

import concourse.bacc as bacc
import concourse.bass as bass
import concourse.mybir as mybir
import concourse.tile as tile
from concourse.masks import make_identity
from concourse.bass_utils import run_bass_kernel_spmd

F32 = mybir.dt.float32
F32R = mybir.dt.float32r
I32 = mybir.dt.int32

N_CORES = 8
D = 128
R = 2
ALPHA = 0.01


# ----------------------------------------------------------------------------
# host-side graph preprocessing
# ----------------------------------------------------------------------------

def _prep(inputs):
    src = np.asarray(inputs["edge_index"][0], dtype=np.int64)
    dst = np.asarray(inputs["edge_index"][1], dtype=np.int64)
    rel = np.asarray(inputs["edge_type"], dtype=np.int64)
    N = int(np.asarray(inputs["des"]).shape[0])
    E = src.shape[0]

    BPC = (-(-N // N_CORES) + 127) // 128  # ceil(ceil(N/8)/128)
    SHARD = BPC * 128
    NBLK = N_CORES * BPC
    TROWS = N_CORES * SHARD

    # per-(dst,rel) counts -> mean weights;  per-dst totals for balancing
    cnt = np.bincount(dst * R + rel, minlength=N * R)
    deg = cnt.reshape(N, R).sum(1)

    # LPT: assign nodes to NBLK blocks (128 slots each) balancing edge load
    order = np.argsort(-deg, kind="stable")
    import heapq
    heap = [(0, b) for b in range(NBLK)]
    heapq.heapify(heap)
    node_block = np.empty(N, np.int64)
    node_lane = np.empty(N, np.int64)
    fill = np.zeros(NBLK, np.int64)
    # process in chunks to cut python overhead: nodes with deg 0 go anywhere
    for n in order:
        load, b = heapq.heappop(heap)
        while fill[b] >= 128:
            load, b = heapq.heappop(heap)
        node_block[n] = b
        node_lane[n] = fill[b]
        fill[b] += 1
        if fill[b] < 128:
            heapq.heappush(heap, (load + int(deg[n]), b))

    node_core = node_block // BPC
    node_pos = (node_block % BPC) * 128 + node_lane      # position in shard
    node_row = node_core * SHARD + node_pos              # row in gather table

    # edge buckets
    e_core = node_core[dst]
    e_block = node_block[dst] % BPC                      # block within core
    key = e_core * BPC + e_block
    bucket_cnt = np.bincount(key, minlength=NBLK)
    T_pad = int(-(-bucket_cnt.max() // 128))

    CAP = T_pad * 128
    order_e = np.argsort(key, kind="stable")
    ks = key[order_e]
    start = np.zeros(NBLK, np.int64)
    start[1:] = np.cumsum(bucket_cnt)[:-1]
    pos_in_bucket = np.arange(E) - start[ks]
    slot = ks * CAP + pos_in_bucket                      # global slot id

    gidx = np.zeros(NBLK * CAP, np.int32)
    cv = np.zeros(NBLK * CAP, np.float32)
    wv = np.zeros(NBLK * CAP, np.float32)
    se, de, re_ = src[order_e], dst[order_e], rel[order_e]
    gidx[slot] = node_row[se].astype(np.int32)
    cv[slot] = (re_ * 128 + node_lane[de]).astype(np.float32)
    wv[slot] = (1.0 / cnt[de * R + re_]).astype(np.float32)

    # reshape to per-core SBUF layouts [128, BPC*T_pad]
    def to_sbuf(a):
        # [NBLK, T_pad, 128] -> per core [128, BPC*T_pad]
        a = a.reshape(N_CORES, BPC, T_pad, 128)
        return np.ascontiguousarray(a.transpose(0, 3, 1, 2).reshape(N_CORES, 128, BPC * T_pad))

    gidx_c = to_sbuf(gidx)
    cv_c = to_sbuf(cv)
    wv_c = to_sbuf(wv)

    # encoder features per core, transposed, in table order
    des = np.asarray(inputs["des"], dtype=np.float32)
    tweet = np.asarray(inputs["tweet"], dtype=np.float32)
    nump = np.asarray(inputs["num_prop"], dtype=np.float32)
    catp = np.asarray(inputs["cat_prop"], dtype=np.float32)
    row_node = np.full(TROWS, -1, np.int64)
    row_node[node_row] = np.arange(N)
    featA = np.zeros((N_CORES, 117, SHARD), np.float32)   # [des; num; cat]
    featB = np.zeros((N_CORES, 100, SHARD), np.float32)   # tweet
    for c in range(N_CORES):
        rows = row_node[c * SHARD:(c + 1) * SHARD]
        m = rows >= 0
        featA[c][0:100, m] = des[rows[m]].T
        featA[c][100:106, m] = nump[rows[m]].T
        featA[c][106:117, m] = catp[rows[m]].T
        featB[c][:, m] = tweet[rows[m]].T

    cfg = dict(N=N, E=E, BPC=BPC, SHARD=SHARD, TROWS=TROWS, T_pad=T_pad)
    per_core = dict(gidx=gidx_c, cv=cv_c, wv=wv_c, featA=featA, featB=featB)
    asm = dict(node_core=node_core, node_pos=node_pos)
    return cfg, per_core, asm


def _weights_inputs(inputs):
    g = lambda k: np.ascontiguousarray(np.asarray(inputs[k], dtype=np.float32))
    w = {}
    WencA = np.zeros((117, 128), np.float32)
    WencA[0:100, 0:32] = g("W_des")
    WencA[100:106, 64:96] = g("W_num")
    WencA[106:117, 96:128] = g("W_cat")
    WencB = np.zeros((100, 128), np.float32)
    WencB[:, 32:64] = g("W_tweet")
    w["WencA"], w["WencB"] = WencA, WencB
    w["Win"], w["Wroot"], w["Wo1"], w["Wo2"] = g("W_in"), g("W_root"), g("W_o1"), g("W_o2")
    wrel = g("W_rel")
    w["Wrel0"], w["Wrel1"] = np.ascontiguousarray(wrel[0]), np.ascontiguousarray(wrel[1])
    w["benc"] = np.concatenate([g("b_des"), g("b_tweet"), g("b_num"), g("b_cat")]).reshape(128, 1)
    w["bin"] = g("b_in").reshape(128, 1)
    w["brg"] = g("b_rgcn").reshape(128, 1)
    w["bo1"] = g("b_o1").reshape(128, 1)
    w["bo2"] = g("b_o2").reshape(2, 1)
    w["iota"] = np.tile(np.arange(256, dtype=np.float32), (128, 1))
    return w


# ----------------------------------------------------------------------------
# device program
# ----------------------------------------------------------------------------

def _enc_slices(shard):
    out, c = [], 0
    while c < shard:
        w = min(512, shard - c)
        out.append((c, w))
        c += w
    return out


def build_bass(cfg, sim_compat=False):
    BPC, SHARD, TROWS, T_pad = cfg["BPC"], cfg["SHARD"], cfg["TROWS"], cfg["T_pad"]
    NT = BPC * T_pad
    nc = bacc.Bacc("TRN2", target_bir_lowering=False, debug=False,
                   num_devices=N_CORES)

    din = lambda n, s: nc.dram_tensor(n, list(s), F32, kind="ExternalInput")
    featA = din("featA", (117, SHARD))
    featB = din("featB", (100, SHARD))
    gidx = nc.dram_tensor("gidx", [128, NT], I32, kind="ExternalInput")
    cvals = din("cvals", (128, NT))
    wvals = din("wvals", (128, NT))
    iota = din("iota", (128, 256))
    WencA, WencB = din("WencA", (117, 128)), din("WencB", (100, 128))
    Win, Wroot = din("Win", (128, 128)), din("Wroot", (128, 128))
    Wrel0, Wrel1 = din("Wrel0", (128, 128)), din("Wrel1", (128, 128))
    Wo1, Wo2 = din("Wo1", (128, 128)), din("Wo2", (128, 2))
    benc, bin_, brg = din("benc", (128, 1)), din("bin", (128, 1)), din("brg", (128, 1))
    bo1, bo2 = din("bo1", (128, 1)), din("bo2", (2, 1))
    out = nc.dram_tensor("out", [2, SHARD], F32, kind="ExternalOutput")

    groups = [list(range(N_CORES))]
    AG = "AllGather"
    BY = mybir.AluOpType.bypass

    def _lrelu(pool, ps_ap, bias_ap, w, name):
        t = pool.tile([ps_ap.shape[0], w], F32R, name=name)
        if not sim_compat:
            nc.scalar.activation(out=t[:], in_=ps_ap,
                                 func=mybir.ActivationFunctionType.Prelu,
                                 bias=bias_ap, scale=1.0, alpha=ALPHA)
            return t
        zt = pool.tile([ps_ap.shape[0], w], F32, name=name + "_z")
        nc.scalar.activation(out=zt[:], in_=ps_ap,
                             func=mybir.ActivationFunctionType.Identity,
                             bias=bias_ap, scale=1.0)
        rt = pool.tile([ps_ap.shape[0], w], F32, name=name + "_r")
        nc.scalar.activation(out=rt[:], in_=ps_ap,
                             func=mybir.ActivationFunctionType.Relu,
                             bias=bias_ap, scale=1.0)
        t1 = pool.tile([ps_ap.shape[0], w], F32, name=name + "_t1")
        nc.vector.tensor_scalar(out=t1[:], in0=zt[:], scalar1=ALPHA, scalar2=None,
                                op0=mybir.AluOpType.mult)
        t2 = pool.tile([ps_ap.shape[0], w], F32, name=name + "_t2")
        nc.vector.tensor_scalar(out=t2[:], in0=rt[:], scalar1=1.0 - ALPHA, scalar2=None,
                                op0=mybir.AluOpType.mult)
        nc.vector.tensor_tensor(out=t[:], in0=t1[:], in1=t2[:],
                                op=mybir.AluOpType.add)
        return t

    with tile.TileContext(nc) as tc:
        with tc.tile_pool(name="const", bufs=1) as cp, \
             tc.tile_pool(name="dram", bufs=1, space="DRAM") as dp:
            # constants
            c_gidx = cp.tile([128, NT], I32); nc.sync.dma_start(c_gidx[:], gidx[:])
            c_cv = cp.tile([128, NT], F32); nc.sync.dma_start(c_cv[:], cvals[:])
            c_wv = cp.tile([128, NT], F32); nc.sync.dma_start(c_wv[:], wvals[:])
            c_iota = cp.tile([128, 256], F32); nc.sync.dma_start(c_iota[:], iota[:])
            rr = lambda ap: ap.bitcast(F32R)
            c_WencA = cp.tile([117, 128], F32R); nc.sync.dma_start(c_WencA[:], rr(WencA[:]))
            c_WencB = cp.tile([100, 128], F32R); nc.sync.dma_start(c_WencB[:], rr(WencB[:]))
            c_Win = cp.tile([128, 128], F32R); nc.sync.dma_start(c_Win[:], rr(Win[:]))
            c_Wroot = cp.tile([128, 128], F32R); nc.sync.dma_start(c_Wroot[:], rr(Wroot[:]))
            c_Wrel0 = cp.tile([128, 128], F32R); nc.sync.dma_start(c_Wrel0[:], rr(Wrel0[:]))
            c_Wrel1 = cp.tile([128, 128], F32R); nc.sync.dma_start(c_Wrel1[:], rr(Wrel1[:]))
            c_Wo1 = cp.tile([128, 128], F32R); nc.sync.dma_start(c_Wo1[:], rr(Wo1[:]))
            c_Wo2 = cp.tile([128, 2], F32R); nc.sync.dma_start(c_Wo2[:], rr(Wo2[:]))
            c_benc = cp.tile([128, 1], F32); nc.sync.dma_start(c_benc[:], benc[:])
            c_bin = cp.tile([128, 1], F32); nc.sync.dma_start(c_bin[:], bin_[:])
            c_brg = cp.tile([128, 1], F32); nc.sync.dma_start(c_brg[:], brg[:])
            c_bo1 = cp.tile([128, 1], F32); nc.sync.dma_start(c_bo1[:], bo1[:])
            c_bo2 = cp.tile([2, 1], F32); nc.sync.dma_start(c_bo2[:], bo2[:])
            ident = cp.tile([128, 128], F32)
            make_identity(nc, ident[:])

            # DRAM intermediates
            xfm = [dp.tile([128, SHARD], F32R, name=f"xfm{i}") for i in range(3)]
            xnm = [dp.tile([SHARD, 128], F32R, name=f"xnm{i}") for i in range(2)]
            tables = [dp.tile([TROWS, 128], F32R, addr_space="Shared", name=f"table{i}")
                      for i in range(2)]

            # ---------------- encoder ----------------
            with tc.tile_pool(name="enc", bufs=3) as ep, \
                 tc.tile_pool(name="encps", bufs=2, space="PSUM") as eps, \
                 tc.tile_pool(name="trps", bufs=2, space="PSUM") as tps:
                for (c0, w) in _enc_slices(SHARD):
                    a_t = ep.tile([117, w], F32R, name="a_t")
                    nc.sync.dma_start(a_t[:], rr(featA[:, c0:c0 + w]))
                    b_t = ep.tile([100, w], F32R, name="b_t")
                    nc.sync.dma_start(b_t[:], rr(featB[:, c0:c0 + w]))
                    ps_e = eps.tile([128, w], F32, name="ps_e")
                    nc.tensor.matmul(out=ps_e[:], lhsT=c_WencA[:], rhs=a_t[:],
                                     start=True, stop=False)
                    nc.tensor.matmul(out=ps_e[:], lhsT=c_WencB[:], rhs=b_t[:],
                                     start=False, stop=True)
                    x0_t = _lrelu(ep, ps_e[:], c_benc[:], w, "x0_t")
                    ps_x = eps.tile([128, w], F32, name="ps_x")
                    nc.tensor.matmul(out=ps_x[:], lhsT=c_Win[:], rhs=x0_t[:],
                                     start=True, stop=True)
                    xf_t = _lrelu(ep, ps_x[:], c_bin[:], w, "xf_t")
                    nc.sync.dma_start(xfm[0][:, c0:c0 + w], xf_t[:])
                    for j in range(w // 128):
                        ps_t = tps.tile([128, 128], F32, name="ps_t")
                        nc.tensor.matmul(out=ps_t[:],
                                         lhsT=xf_t[:, j * 128:(j + 1) * 128].bitcast(F32),
                                         rhs=ident[:], is_transpose=True,
                                         start=True, stop=True)
                        tr_t = ep.tile([128, 128], F32R, name="tr_t")
                        nc.vector.tensor_copy(out=tr_t[:], in_=ps_t[:])
                        nc.sync.dma_start(xnm[0][c0 + j * 128:c0 + (j + 1) * 128, :], tr_t[:])

            nc.gpsimd.collective_compute(AG, BY, replica_groups=groups,
                                         ins=[xnm[0].opt()], outs=[tables[0].opt()])

            # ---------------- rgcn layers ----------------
            for L in range(2):
                table, xin, xout = tables[L], xfm[L], xfm[L + 1]
                with tc.tile_pool(name=f"gp{L}", bufs=16) as gp, \
                     tc.tile_pool(name=f"sp{L}", bufs=8) as sp, \
                     tc.tile_pool(name=f"up{L}", bufs=2) as up, \
                     tc.tile_pool(name=f"Sps{L}", bufs=4, space="PSUM") as Sps, \
                     tc.tile_pool(name=f"aps{L}", bufs=2, space="PSUM") as aps, \
                     tc.tile_pool(name=f"tps{L}", bufs=2, space="PSUM") as tps:
                    n_units = BPC // 2
                    for u in range(n_units):
                        psS = []
                        for h in range(2):
                            b = u * 2 + h
                            ps = Sps.tile([128, 256], F32, name="psS")
                            psS.append(ps)
                            for t in range(T_pad):
                                T = b * T_pad + t
                                G = gp.tile([128, 128], F32R, name="G")
                                nc.gpsimd.indirect_dma_start(
                                    out=G[:], out_offset=None, in_=table[:],
                                    in_offset=bass.IndirectOffsetOnAxis(
                                        ap=c_gidx[:, T:T + 1], axis=0))
                                sel = sp.tile([128, 256], F32R, name="sel")
                                nc.vector.tensor_scalar(
                                    out=sel[:], in0=c_iota[:],
                                    scalar1=c_cv[:, T:T + 1], scalar2=c_wv[:, T:T + 1],
                                    op0=mybir.AluOpType.is_equal,
                                    op1=mybir.AluOpType.mult)
                                nc.tensor.matmul(out=ps[:], lhsT=G[:], rhs=sel[:],
                                                 start=(t == 0), stop=(t == T_pad - 1))
                        # unit tail: transforms for 2 blocks (256 dst cols)
                        U0 = up.tile([128, 256], F32R, name="U0")
                        U1 = up.tile([128, 256], F32R, name="U1")
                        for h in range(2):
                            nc.vector.tensor_copy(out=U0[:, h * 128:(h + 1) * 128],
                                                  in_=psS[h][:, 0:128])
                            nc.vector.tensor_copy(out=U1[:, h * 128:(h + 1) * 128],
                                                  in_=psS[h][:, 128:256])
                        xr = up.tile([128, 256], F32R, name="xr")
                        nc.sync.dma_start(xr[:], xin[:, u * 256:(u + 1) * 256])
                        agg = aps.tile([128, 256], F32, name="agg")
                        nc.tensor.matmul(out=agg[:], lhsT=c_Wroot[:], rhs=xr[:],
                                         start=True, stop=False)
                        nc.tensor.matmul(out=agg[:], lhsT=c_Wrel0[:], rhs=U0[:],
                                         start=False, stop=False)
                        nc.tensor.matmul(out=agg[:], lhsT=c_Wrel1[:], rhs=U1[:],
                                         start=False, stop=True)
                        y = up.tile([128, 256], F32R, name="y")
                        nc.scalar.activation(out=y[:], in_=agg[:],
                                             func=mybir.ActivationFunctionType.Identity,
                                             bias=c_brg[:], scale=1.0)
                        nc.sync.dma_start(xout[:, u * 256:(u + 1) * 256], y[:])
                        if L == 0:
                            for j in range(2):
                                ps_t = tps.tile([128, 128], F32, name="ps_t2")
                                nc.tensor.matmul(
                                    out=ps_t[:],
                                    lhsT=y[:, j * 128:(j + 1) * 128].bitcast(F32),
                                    rhs=ident[:], is_transpose=True,
                                    start=True, stop=True)
                                tr_t = up.tile([128, 128], F32R, name="tr2")
                                nc.vector.tensor_copy(out=tr_t[:], in_=ps_t[:])
                                nc.sync.dma_start(
                                    xnm[1][u * 256 + j * 128:u * 256 + (j + 1) * 128, :],
                                    tr_t[:])
                if L == 0:
                    nc.gpsimd.collective_compute(AG, BY, replica_groups=groups,
                                                 ins=[xnm[1].opt()],
                                                 outs=[tables[1].opt()])

            # ---------------- head ----------------
            with tc.tile_pool(name="hd", bufs=3) as hp, \
                 tc.tile_pool(name="hps", bufs=2, space="PSUM") as hps:
                for (c0, w) in _enc_slices(SHARD):
                    xt = hp.tile([128, w], F32R, name="xt")
                    nc.sync.dma_start(xt[:], xfm[2][:, c0:c0 + w])
                    ps_h = hps.tile([128, w], F32, name="ps_h")
                    nc.tensor.matmul(out=ps_h[:], lhsT=c_Wo1[:], rhs=xt[:],
                                     start=True, stop=True)
                    z_t = _lrelu(hp, ps_h[:], c_bo1[:], w, "z_t")
                    ps_o = hps.tile([2, w], F32, name="ps_o")
                    nc.tensor.matmul(out=ps_o[:], lhsT=c_Wo2[:], rhs=z_t[:],
                                     start=True, stop=True)
                    o_t = hp.tile([2, w], F32, name="o_t")
                    nc.scalar.activation(out=o_t[:], in_=ps_o[:],
                                         func=mybir.ActivationFunctionType.Identity,
                                         bias=c_bo2[:], scale=1.0)
                    nc.sync.dma_start(out[:, c0:c0 + w], o_t[:])
    nc.compile()
    return nc


# ----------------------------------------------------------------------------
# entry point
# ----------------------------------------------------------------------------

def _in_maps(cfg, per_core, w):
    maps = []
    for c in range(N_CORES):
        m = dict(featA=per_core["featA"][c], featB=per_core["featB"][c],
                 gidx=per_core["gidx"][c], cvals=per_core["cv"][c],
                 wvals=per_core["wv"][c])
        m.update({k: w[k] for k in ("WencA", "WencB", "Win",
                                    "Wroot", "Wrel0", "Wrel1", "Wo1", "Wo2",
                                    "benc", "bin", "brg", "bo1", "bo2", "iota")})
        maps.append(m)
    return maps


def _assemble(cfg, asm, core_outs):
    N = cfg["N"]
    stacked = np.stack([co["out"] for co in core_outs])      # [8, 2, SHARD]
    out = stacked[asm["node_core"], :, asm["node_pos"]]       # [N, 2]
    return np.ascontiguousarray(out.astype(np.float32))


_NC_CACHE = {}


def kernel(**inputs):
    cfg, per_core, asm = _prep(inputs)
    w = _weights_inputs(inputs)
    key = (cfg["N"], cfg["E"], cfg["T_pad"])
    nc = _NC_CACHE.get(key)
    if nc is None:
        nc = build_bass(cfg)
        _NC_CACHE[key] = nc
    maps = _in_maps(cfg, per_core, w)
    res = run_bass_kernel_spmd(nc, maps, core_ids=list(range(N_CORES)))
    return _assemble(cfg, asm, res.results)



# revision 5
# speedup vs baseline: 6.0038x; 6.0038x over previous
"""BotRGCN on 8 Trainium2 NeuronCores (Bass/Tile).

Strategy (1-D destination-sharded graph partition):
  - Host assigns nodes to 8*BPC blocks of 128 destinations via sorted-serpentine
    balancing on in-degree, so every block holds ~E/(8*BPC) edges; shard s =
    blocks [s*BPC, (s+1)*BPC).  Edges are bucketed per (core, dst-block) and
    padded to a uniform T_pad tiles of 128 edges.
  - Each core: encoder (feature-major matmuls, bf16, fused Prelu) ->
    node-major x via PE transposes -> AllGather into a replicated bf16 table ->
    per dst-block: per-tile indirect-DMA row gather + DVE weighted one-hot
    (tensor_scalar is_equal*mult) + scatter-matmul accumulating
    relation-split sums in PSUM (f32) -> per 2-block unit: W_rel / W_root
    transform matmuls + bias -> layer output (feature-major) and next gather
    table.  Layer 2 identical; head = two matmuls + Prelu; per-core output
    [2, SHARD], host inverse-permutes to [N, 2].

  Wall-clock is dominated by host->device transfer over the tunneled PJRT
  link, so the data plane is bf16 (features optionally int8 with the
  dequant scale folded into the encoder weights) and the compiled
  executable + jit trace are cached across calls.
"""

import numpy as np
import ml_dtypes

import jax
from jax.sharding import Mesh, PartitionSpec
from jax.experimental.shard_map import shard_map

import concourse.bacc as bacc
import concourse.bass as bass
import concourse.bass2jax as b2j
import concourse.mybir as mybir
import concourse.tile as tile
from concourse.masks import make_identity

F32 = mybir.dt.float32
BF16 = mybir.dt.bfloat16
I32 = mybir.dt.int32
I8 = mybir.dt.int8
BF = ml_dtypes.bfloat16

N_CORES = 8
D = 128
R = 2
ALPHA = 0.01

FEAT_INT8 = True            # ship encoder features as int8 (scale folded into W)
QSCALE = 4.5 / 127.0        # int8 quant step for N(0,1) features


# ----------------------------------------------------------------------------
# host-side graph preprocessing (fully vectorized)
# ----------------------------------------------------------------------------

def _prep(inputs):
    src = np.asarray(inputs["edge_index"][0], dtype=np.int64)
    dst = np.asarray(inputs["edge_index"][1], dtype=np.int64)
    rel = np.asarray(inputs["edge_type"], dtype=np.int64)
    N = int(np.asarray(inputs["des"]).shape[0])
    E = src.shape[0]

    BPC = (-(-N // N_CORES) + 127) // 128  # ceil(ceil(N/8)/128)
    SHARD = BPC * 128
    NBLK = N_CORES * BPC
    TROWS = N_CORES * SHARD
    assert N <= NBLK * 128

    # per-(dst,rel) counts -> mean weights;  per-dst totals for balancing
    cnt = np.bincount(dst * R + rel, minlength=N * R)
    deg = cnt.reshape(N, R).sum(1)

    # sorted-serpentine: nodes by degree desc, dealt across NBLK blocks
    # alternating direction each round -> near-optimal edge balance.
    order = np.argsort(-deg, kind="stable")
    idx = np.arange(N)
    rnd = idx // NBLK
    pos = idx % NBLK
    blk = np.where(rnd % 2 == 0, pos, NBLK - 1 - pos)
    node_block = np.empty(N, np.int64)
    node_lane = np.empty(N, np.int64)
    node_block[order] = blk
    node_lane[order] = rnd

    node_core = node_block // BPC
    node_pos = (node_block % BPC) * 128 + node_lane      # position in shard
    node_row = node_core * SHARD + node_pos              # row in gather table

    # edge buckets keyed by destination block
    key = node_block[dst]
    bucket_cnt = np.bincount(key, minlength=NBLK)
    T_pad = int(-(-bucket_cnt.max() // 128))

    CAP = T_pad * 128
    order_e = np.argsort(key, kind="stable")
    ks = key[order_e]
    start = np.zeros(NBLK, np.int64)
    start[1:] = np.cumsum(bucket_cnt)[:-1]
    pos_in_bucket = np.arange(E) - start[ks]
    slot = ks * CAP + pos_in_bucket                      # global slot id

    gidx = np.zeros(NBLK * CAP, np.int32)
    cv = np.zeros(NBLK * CAP, np.float32)
    wv = np.zeros(NBLK * CAP, np.float32)
    se, de, re_ = src[order_e], dst[order_e], rel[order_e]
    gidx[slot] = node_row[se].astype(np.int32)
    cv[slot] = (re_ * 128 + node_lane[de]).astype(np.float32)
    wv[slot] = (1.0 / cnt[de * R + re_]).astype(np.float32)

    # reshape to per-core SBUF layouts [128, BPC*T_pad]
    def to_sbuf(a):
        # [NBLK, T_pad, 128] -> per core [128, BPC*T_pad]
        a = a.reshape(N_CORES, BPC, T_pad, 128)
        return np.ascontiguousarray(a.transpose(0, 3, 1, 2).reshape(N_CORES, 128, BPC * T_pad))

    gidx_c = to_sbuf(gidx)
    cv_c = to_sbuf(cv).astype(BF)      # integers < 256: exact in bf16
    wv_c = to_sbuf(wv).astype(BF)

    # encoder features per core, transposed, in table order
    des = np.asarray(inputs["des"], dtype=np.float32)
    tweet = np.asarray(inputs["tweet"], dtype=np.float32)
    nump = np.asarray(inputs["num_prop"], dtype=np.float32)
    catp = np.asarray(inputs["cat_prop"], dtype=np.float32)
    row_node = np.full(TROWS, -1, np.int64)
    row_node[node_row] = np.arange(N)
    valid = row_node >= 0
    safe = np.where(valid, row_node, 0)
    F = np.concatenate([des, nump, catp, tweet], axis=1)[safe]   # [TROWS, 217]
    F[~valid] = 0.0
    if FEAT_INT8:
        Fq = np.clip(np.rint(F * (1.0 / QSCALE)), -127, 127).astype(np.int8)
    else:
        Fq = F.astype(BF)
    Fq = np.ascontiguousarray(
        Fq.reshape(N_CORES, SHARD, 217).transpose(0, 2, 1))     # [8, 217, SHARD]
    featA = Fq[:, 0:117]     # [des; num; cat]
    featB = Fq[:, 117:217]   # tweet

    cfg = dict(N=N, E=E, BPC=BPC, SHARD=SHARD, TROWS=TROWS, T_pad=T_pad)
    per_core = dict(gidx=gidx_c, cv=cv_c, wv=wv_c, featA=featA, featB=featB)
    asm = dict(node_core=node_core, node_pos=node_pos)
    return cfg, per_core, asm


def _weights_inputs(inputs):
    g = lambda k: np.asarray(inputs[k], dtype=np.float32)
    s = QSCALE if FEAT_INT8 else 1.0
    w = {}
    WencA = np.zeros((117, 128), np.float32)
    WencA[0:100, 0:32] = g("W_des")
    WencA[100:106, 64:96] = g("W_num")
    WencA[106:117, 96:128] = g("W_cat")
    WencB = np.zeros((100, 128), np.float32)
    WencB[:, 32:64] = g("W_tweet")
    w["WencA"], w["WencB"] = (WencA * s).astype(BF), (WencB * s).astype(BF)
    for k, src in (("Win", "W_in"), ("Wroot", "W_root"), ("Wo1", "W_o1"),
                   ("Wo2", "W_o2")):
        w[k] = g(src).astype(BF)
    wrel = g("W_rel")
    w["Wrel0"] = np.ascontiguousarray(wrel[0]).astype(BF)
    w["Wrel1"] = np.ascontiguousarray(wrel[1]).astype(BF)
    w["benc"] = np.concatenate([g("b_des"), g("b_tweet"), g("b_num"),
                                g("b_cat")]).reshape(128, 1)
    w["bin"] = g("b_in").reshape(128, 1)
    w["brg"] = g("b_rgcn").reshape(128, 1)
    w["bo1"] = g("b_o1").reshape(128, 1)
    w["bo2"] = g("b_o2").reshape(2, 1)
    w["iota"] = np.tile(np.arange(256, dtype=np.float32), (128, 1))
    return w


# ----------------------------------------------------------------------------
# device program
# ----------------------------------------------------------------------------

def _enc_slices(shard):
    out, c = [], 0
    while c < shard:
        w = min(512, shard - c)
        out.append((c, w))
        c += w
    return out


def build_bass(cfg, sim_compat=False):
    BPC, SHARD, TROWS, T_pad = cfg["BPC"], cfg["SHARD"], cfg["TROWS"], cfg["T_pad"]
    NT = BPC * T_pad
    nc = bacc.Bacc("TRN2", target_bir_lowering=False, debug=False,
                   num_devices=N_CORES)

    FEAT = I8 if FEAT_INT8 else BF16
    din = lambda n, s, d: nc.dram_tensor(n, list(s), d, kind="ExternalInput")
    featA = din("featA", (117, SHARD), FEAT)
    featB = din("featB", (100, SHARD), FEAT)
    gidx = din("gidx", (128, NT), I32)
    cvals = din("cvals", (128, NT), BF16)
    wvals = din("wvals", (128, NT), BF16)
    iota = din("iota", (128, 256), F32)
    WencA, WencB = din("WencA", (117, 128), BF16), din("WencB", (100, 128), BF16)
    Win, Wroot = din("Win", (128, 128), BF16), din("Wroot", (128, 128), BF16)
    Wrel0, Wrel1 = din("Wrel0", (128, 128), BF16), din("Wrel1", (128, 128), BF16)
    Wo1, Wo2 = din("Wo1", (128, 128), BF16), din("Wo2", (128, 2), BF16)
    benc, bin_, brg = (din("benc", (128, 1), F32), din("bin", (128, 1), F32),
                       din("brg", (128, 1), F32))
    bo1, bo2 = din("bo1", (128, 1), F32), din("bo2", (2, 1), F32)
    out = nc.dram_tensor("out", [2, SHARD], F32, kind="ExternalOutput")

    groups = [list(range(N_CORES))]
    AG = "AllGather"
    BY = mybir.AluOpType.bypass

    def _lrelu(pool, ps_ap, bias_ap, w, name):
        t = pool.tile([ps_ap.shape[0], w], BF16, name=name)
        if not sim_compat:
            nc.scalar.activation(out=t[:], in_=ps_ap,
                                 func=mybir.ActivationFunctionType.Prelu,
                                 bias=bias_ap, scale=1.0, alpha=ALPHA)
            return t
        zt = pool.tile([ps_ap.shape[0], w], F32, name=name + "_z")
        nc.scalar.activation(out=zt[:], in_=ps_ap,
                             func=mybir.ActivationFunctionType.Identity,
                             bias=bias_ap, scale=1.0)
        rt = pool.tile([ps_ap.shape[0], w], F32, name=name + "_r")
        nc.scalar.activation(out=rt[:], in_=ps_ap,
                             func=mybir.ActivationFunctionType.Relu,
                             bias=bias_ap, scale=1.0)
        t1 = pool.tile([ps_ap.shape[0], w], F32, name=name + "_t1")
        nc.vector.tensor_scalar(out=t1[:], in0=zt[:], scalar1=ALPHA, scalar2=None,
                                op0=mybir.AluOpType.mult)
        t2 = pool.tile([ps_ap.shape[0], w], F32, name=name + "_t2")
        nc.vector.tensor_scalar(out=t2[:], in0=rt[:], scalar1=1.0 - ALPHA, scalar2=None,
                                op0=mybir.AluOpType.mult)
        nc.vector.tensor_tensor(out=t[:], in0=t1[:], in1=t2[:],
                                op=mybir.AluOpType.add)
        return t

    with tile.TileContext(nc) as tc:
        with tc.tile_pool(name="const", bufs=1) as cp, \
             tc.tile_pool(name="dram", bufs=1, space="DRAM") as dp:
            # constants
            c_gidx = cp.tile([128, NT], I32); nc.sync.dma_start(c_gidx[:], gidx[:])
            c_cv16 = cp.tile([128, NT], BF16); nc.sync.dma_start(c_cv16[:], cvals[:])
            c_wv16 = cp.tile([128, NT], BF16); nc.sync.dma_start(c_wv16[:], wvals[:])
            c_cv = cp.tile([128, NT], F32)
            nc.vector.tensor_copy(out=c_cv[:], in_=c_cv16[:])
            c_wv = cp.tile([128, NT], F32)
            nc.vector.tensor_copy(out=c_wv[:], in_=c_wv16[:])
            c_iota = cp.tile([128, 256], F32); nc.sync.dma_start(c_iota[:], iota[:])
            c_WencA = cp.tile([117, 128], BF16); nc.sync.dma_start(c_WencA[:], WencA[:])
            c_WencB = cp.tile([100, 128], BF16); nc.sync.dma_start(c_WencB[:], WencB[:])
            c_Win = cp.tile([128, 128], BF16); nc.sync.dma_start(c_Win[:], Win[:])
            c_Wroot = cp.tile([128, 128], BF16); nc.sync.dma_start(c_Wroot[:], Wroot[:])
            c_Wrel0 = cp.tile([128, 128], BF16); nc.sync.dma_start(c_Wrel0[:], Wrel0[:])
            c_Wrel1 = cp.tile([128, 128], BF16); nc.sync.dma_start(c_Wrel1[:], Wrel1[:])
            c_Wo1 = cp.tile([128, 128], BF16); nc.sync.dma_start(c_Wo1[:], Wo1[:])
            c_Wo2 = cp.tile([128, 2], BF16); nc.sync.dma_start(c_Wo2[:], Wo2[:])
            c_benc = cp.tile([128, 1], F32); nc.sync.dma_start(c_benc[:], benc[:])
            c_bin = cp.tile([128, 1], F32); nc.sync.dma_start(c_bin[:], bin_[:])
            c_brg = cp.tile([128, 1], F32); nc.sync.dma_start(c_brg[:], brg[:])
            c_bo1 = cp.tile([128, 1], F32); nc.sync.dma_start(c_bo1[:], bo1[:])
            c_bo2 = cp.tile([2, 1], F32); nc.sync.dma_start(c_bo2[:], bo2[:])
            ident = cp.tile([128, 128], BF16)
            make_identity(nc, ident[:])

            # DRAM intermediates
            xfm = [dp.tile([128, SHARD], BF16, name=f"xfm{i}") for i in range(3)]
            xnm = [dp.tile([SHARD, 128], BF16, name=f"xnm{i}") for i in range(2)]
            tables = [dp.tile([TROWS, 128], BF16, addr_space="Shared", name=f"table{i}")
                      for i in range(2)]

            # ---------------- encoder ----------------
            with tc.tile_pool(name="enc", bufs=3) as ep, \
                 tc.tile_pool(name="encps", bufs=2, space="PSUM") as eps, \
                 tc.tile_pool(name="trps", bufs=2, space="PSUM") as tps:
                for (c0, w) in _enc_slices(SHARD):
                    a_t = ep.tile([117, w], FEAT, name="a_t")
                    nc.sync.dma_start(a_t[:], featA[:, c0:c0 + w])
                    b_t = ep.tile([100, w], FEAT, name="b_t")
                    nc.sync.dma_start(b_t[:], featB[:, c0:c0 + w])
                    if FEAT_INT8:
                        a_c = ep.tile([117, w], BF16, name="a_c")
                        nc.vector.tensor_copy(out=a_c[:], in_=a_t[:])
                        b_c = ep.tile([100, w], BF16, name="b_c")
                        nc.vector.tensor_copy(out=b_c[:], in_=b_t[:])
                    else:
                        a_c, b_c = a_t, b_t
                    ps_e = eps.tile([128, w], F32, name="ps_e")
                    nc.tensor.matmul(out=ps_e[:], lhsT=c_WencA[:], rhs=a_c[:],
                                     start=True, stop=False)
                    nc.tensor.matmul(out=ps_e[:], lhsT=c_WencB[:], rhs=b_c[:],
                                     start=False, stop=True)
                    x0_t = _lrelu(ep, ps_e[:], c_benc[:], w, "x0_t")
                    ps_x = eps.tile([128, w], F32, name="ps_x")
                    nc.tensor.matmul(out=ps_x[:], lhsT=c_Win[:], rhs=x0_t[:],
                                     start=True, stop=True)
                    xf_t = _lrelu(ep, ps_x[:], c_bin[:], w, "xf_t")
                    nc.sync.dma_start(xfm[0][:, c0:c0 + w], xf_t[:])
                    for j in range(w // 128):
                        ps_t = tps.tile([128, 128], BF16, name="ps_t")
                        nc.tensor.matmul(out=ps_t[:],
                                         lhsT=xf_t[:, j * 128:(j + 1) * 128],
                                         rhs=ident[:], is_transpose=True,
                                         start=True, stop=True)
                        tr_t = ep.tile([128, 128], BF16, name="tr_t")
                        nc.vector.tensor_copy(out=tr_t[:], in_=ps_t[:])
                        nc.sync.dma_start(xnm[0][c0 + j * 128:c0 + (j + 1) * 128, :], tr_t[:])

            nc.gpsimd.collective_compute(AG, BY, replica_groups=groups,
                                         ins=[xnm[0].opt()], outs=[tables[0].opt()])

            # ---------------- rgcn layers ----------------
            for L in range(2):
                table, xin, xout = tables[L], xfm[L], xfm[L + 1]
                with tc.tile_pool(name=f"gp{L}", bufs=16) as gp, \
                     tc.tile_pool(name=f"sp{L}", bufs=8) as sp, \
                     tc.tile_pool(name=f"up{L}", bufs=2) as up, \
                     tc.tile_pool(name=f"Sps{L}", bufs=4, space="PSUM") as Sps, \
                     tc.tile_pool(name=f"aps{L}", bufs=2, space="PSUM") as aps, \
                     tc.tile_pool(name=f"tps{L}", bufs=2, space="PSUM") as tps:
                    n_units = BPC // 2
                    for u in range(n_units):
                        psS = []
                        for h in range(2):
                            b = u * 2 + h
                            ps = Sps.tile([128, 256], F32, name="psS")
                            psS.append(ps)
                            for t in range(T_pad):
                                T = b * T_pad + t
                                G = gp.tile([128, 128], BF16, name="G")
                                nc.gpsimd.indirect_dma_start(
                                    out=G[:], out_offset=None, in_=table[:],
                                    in_offset=bass.IndirectOffsetOnAxis(
                                        ap=c_gidx[:, T:T + 1], axis=0))
                                sel = sp.tile([128, 256], BF16, name="sel")
                                nc.vector.tensor_scalar(
                                    out=sel[:], in0=c_iota[:],
                                    scalar1=c_cv[:, T:T + 1], scalar2=c_wv[:, T:T + 1],
                                    op0=mybir.AluOpType.is_equal,
                                    op1=mybir.AluOpType.mult)
                                nc.tensor.matmul(out=ps[:], lhsT=G[:], rhs=sel[:],
                                                 start=(t == 0), stop=(t == T_pad - 1))
                        # unit tail: transforms for 2 blocks (256 dst cols)
                        U0 = up.tile([128, 256], BF16, name="U0")
                        U1 = up.tile([128, 256], BF16, name="U1")
                        for h in range(2):
                            nc.vector.tensor_copy(out=U0[:, h * 128:(h + 1) * 128],
                                                  in_=psS[h][:, 0:128])
                            nc.vector.tensor_copy(out=U1[:, h * 128:(h + 1) * 128],
                                                  in_=psS[h][:, 128:256])
                        xr = up.tile([128, 256], BF16, name="xr")
                        nc.sync.dma_start(xr[:], xin[:, u * 256:(u + 1) * 256])
                        agg = aps.tile([128, 256], F32, name="agg")
                        nc.tensor.matmul(out=agg[:], lhsT=c_Wroot[:], rhs=xr[:],
                                         start=True, stop=False)
                        nc.tensor.matmul(out=agg[:], lhsT=c_Wrel0[:], rhs=U0[:],
                                         start=False, stop=False)
                        nc.tensor.matmul(out=agg[:], lhsT=c_Wrel1[:], rhs=U1[:],
                                         start=False, stop=True)
                        y = up.tile([128, 256], BF16, name="y")
                        nc.scalar.activation(out=y[:], in_=agg[:],
                                             func=mybir.ActivationFunctionType.Identity,
                                             bias=c_brg[:], scale=1.0)
                        nc.sync.dma_start(xout[:, u * 256:(u + 1) * 256], y[:])
                        if L == 0:
                            for j in range(2):
                                ps_t = tps.tile([128, 128], BF16, name="ps_t2")
                                nc.tensor.matmul(
                                    out=ps_t[:],
                                    lhsT=y[:, j * 128:(j + 1) * 128],
                                    rhs=ident[:], is_transpose=True,
                                    start=True, stop=True)
                                tr_t = up.tile([128, 128], BF16, name="tr2")
                                nc.vector.tensor_copy(out=tr_t[:], in_=ps_t[:])
                                nc.sync.dma_start(
                                    xnm[1][u * 256 + j * 128:u * 256 + (j + 1) * 128, :],
                                    tr_t[:])
                if L == 0:
                    nc.gpsimd.collective_compute(AG, BY, replica_groups=groups,
                                                 ins=[xnm[1].opt()],
                                                 outs=[tables[1].opt()])

            # ---------------- head ----------------
            with tc.tile_pool(name="hd", bufs=3) as hp, \
                 tc.tile_pool(name="hps", bufs=2, space="PSUM") as hps:
                for (c0, w) in _enc_slices(SHARD):
                    xt = hp.tile([128, w], BF16, name="xt")
                    nc.sync.dma_start(xt[:], xfm[2][:, c0:c0 + w])
                    ps_h = hps.tile([128, w], F32, name="ps_h")
                    nc.tensor.matmul(out=ps_h[:], lhsT=c_Wo1[:], rhs=xt[:],
                                     start=True, stop=True)
                    z_t = _lrelu(hp, ps_h[:], c_bo1[:], w, "z_t")
                    ps_o = hps.tile([2, w], F32, name="ps_o")
                    nc.tensor.matmul(out=ps_o[:], lhsT=c_Wo2[:], rhs=z_t[:],
                                     start=True, stop=True)
                    o_t = hp.tile([2, w], F32, name="o_t")
                    nc.scalar.activation(out=o_t[:], in_=ps_o[:],
                                         func=mybir.ActivationFunctionType.Identity,
                                         bias=c_bo2[:], scale=1.0)
                    nc.sync.dma_start(out[:, c0:c0 + w], o_t[:])
    nc.compile()
    return nc


# ----------------------------------------------------------------------------
# cached PJRT runner (jit trace + NEFF compile + device load happen once)
# ----------------------------------------------------------------------------

class _Runner:
    def __init__(self, cfg):
        self.cfg = cfg
        self.nc = build_bass(cfg)
        b2j.install_neuronx_cc_hook()
        nc = self.nc
        partition_name = (nc.partition_id_tensor.name
                          if nc.partition_id_tensor else None)
        in_names, out_names, out_avals = [], [], []
        for alloc in nc.m.functions[0].allocations:
            if not isinstance(alloc, mybir.MemoryLocationSet):
                continue
            name = alloc.memorylocations[0].name
            if alloc.kind == "ExternalInput":
                if name != partition_name:
                    in_names.append(name)
            elif alloc.kind == "ExternalOutput":
                shape = tuple(alloc.tensor_shape)
                dtype = mybir.dt.np(alloc.dtype)
                out_names.append(name)
                out_avals.append(jax.core.ShapedArray(shape, dtype))
        self.in_names = list(in_names)
        self.out_names = out_names
        self.out_avals = out_avals
        n_params = len(in_names)
        n_outs = len(out_avals)
        bind_names = in_names + out_names
        if partition_name is not None:
            bind_names = bind_names + [partition_name]

        def _body(*args):
            operands = list(args)
            if partition_name is not None:
                operands.append(b2j.partition_id_tensor())
            outs = b2j._bass_exec_p.bind(
                *operands,
                out_avals=tuple(out_avals),
                in_names=tuple(bind_names),
                out_names=tuple(out_names),
                lowering_input_output_aliases=(),
                sim_require_finite=True,
                sim_require_nnan=True,
                nc=nc,
            )
            return tuple(outs)

        devices = jax.devices()[:N_CORES]
        mesh = Mesh(np.asarray(devices), ("core",))
        in_specs = (PartitionSpec("core"),) * (n_params + n_outs)
        out_specs = (PartitionSpec("core"),) * n_outs
        self.sharded = jax.jit(
            shard_map(_body, mesh=mesh, in_specs=in_specs, out_specs=out_specs,
                      check_rep=False),
            donate_argnums=tuple(range(n_params, n_params + n_outs)),
            keep_unused=True,
        )

    def __call__(self, maps):
        concat_in = [
            np.concatenate([np.asarray(m[n]) for m in maps], axis=0)
            for n in self.in_names
        ]
        concat_zeros = [
            np.zeros((N_CORES * a.shape[0], *a.shape[1:]), a.dtype)
            for a in self.out_avals
        ]
        outs = self.sharded(*concat_in, *concat_zeros)
        return [
            {name: np.asarray(outs[i]).reshape(N_CORES, *self.out_avals[i].shape)[c]
             for i, name in enumerate(self.out_names)}
            for c in range(N_CORES)
        ]


_RUNNERS = {}


def _get_runner(cfg):
    key = (cfg["N"], cfg["E"], cfg["T_pad"], FEAT_INT8)
    r = _RUNNERS.get(key)
    if r is None:
        r = _Runner(cfg)
        _RUNNERS[key] = r
    return r


# ----------------------------------------------------------------------------
# entry point
# ----------------------------------------------------------------------------

def _in_maps(cfg, per_core, w):
    maps = []
    for c in range(N_CORES):
        m = dict(featA=per_core["featA"][c], featB=per_core["featB"][c],
                 gidx=per_core["gidx"][c], cvals=per_core["cv"][c],
                 wvals=per_core["wv"][c])
        m.update({k: w[k] for k in ("WencA", "WencB", "Win",
                                    "Wroot", "Wrel0", "Wrel1", "Wo1", "Wo2",
                                    "benc", "bin", "brg", "bo1", "bo2", "iota")})
        maps.append(m)
    return maps


def _assemble(cfg, asm, core_outs):
    stacked = np.stack([co["out"] for co in core_outs])      # [8, 2, SHARD]
    out = stacked[asm["node_core"], :, asm["node_pos"]]       # [N, 2]
    return np.ascontiguousarray(out.astype(np.float32))


def kernel(**inputs):
    cfg, per_core, asm = _prep(inputs)
    w = _weights_inputs(inputs)
    runner = _get_runner(cfg)
    maps = _in_maps(cfg, per_core, w)
    res = runner(maps)
    return _assemble(cfg, asm, res)


# revision 7
# speedup vs baseline: 8.7277x; 1.4537x over previous
"""BotRGCN on 8 Trainium2 NeuronCores (Bass/Tile).

Strategy (1-D destination-sharded graph partition):
  - Host assigns nodes to 8*BPC blocks of 128 destinations via sorted-serpentine
    balancing on in-degree, so every block holds ~E/(8*BPC) edges; shard s =
    blocks [s*BPC, (s+1)*BPC).  Edges are bucketed per (core, dst-block) and
    padded to a uniform T_pad tiles of 128 edges.
  - The tiny encoder MLP (4.7 GFLOP) runs on host in f32; nodes ship as the
    128-d embedding x quantized int8 with per-feature scales (the wall-clock
    is dominated by the tunneled host->device link, so wire bytes rule).
  - Each core: int8 x -> bf16 (raw ints) -> AllGather into a replicated bf16
    gather table; PE transposes build the feature-major copy with the
    dequant scale applied per partition.  Per dst-block: per-tile
    indirect-DMA row gather + DVE weighted one-hot (tensor_scalar
    is_equal*mult) + scatter-matmul accumulating relation-split sums in
    PSUM (f32); per 2-block unit: W_rel / W_root transform matmuls + bias.
    Layer-1 U copies apply the int8 dequant scale; layer 2 runs true-scale.
    Head = two matmuls + Prelu; per-core output [2, SHARD], host
    inverse-permutes to [N, 2].
  - Edge metadata ships as ONE int32 per edge slot: gather row (17 bits) |
    dst (lane,rel) selector cv (9 bits, 511 = padding sentinel); mean
    weights 1/cnt ship bf16.  The compiled executable + jit trace are
    cached across calls, so each call pays only transfer + execute.
"""

import numpy as np
import ml_dtypes

import jax
from jax.sharding import Mesh, PartitionSpec
from jax.experimental.shard_map import shard_map

import concourse.bacc as bacc
import concourse.bass as bass
import concourse.bass2jax as b2j
import concourse.mybir as mybir
import concourse.tile as tile
from concourse.masks import make_identity

F32 = mybir.dt.float32
BF16 = mybir.dt.bfloat16
I32 = mybir.dt.int32
I8 = mybir.dt.int8
BF = ml_dtypes.bfloat16

N_CORES = 8
D = 128
R = 2
ALPHA = 0.01


# ----------------------------------------------------------------------------
# host-side preprocessing (vectorized): graph partition + encoder + quant
# ----------------------------------------------------------------------------

def _lrelu_np(v):
    return np.where(v > 0, v, np.float32(ALPHA) * v)


def _prep(inputs):
    src = np.asarray(inputs["edge_index"][0], dtype=np.int64)
    dst = np.asarray(inputs["edge_index"][1], dtype=np.int64)
    rel = np.asarray(inputs["edge_type"], dtype=np.int64)
    N = int(np.asarray(inputs["des"]).shape[0])
    E = src.shape[0]

    BPC = (-(-N // N_CORES) + 127) // 128  # ceil(ceil(N/8)/128)
    SHARD = BPC * 128
    NBLK = N_CORES * BPC
    TROWS = N_CORES * SHARD
    assert N <= NBLK * 128
    assert TROWS <= (1 << 17), "gather row must fit 17 bits"

    # per-(dst,rel) counts -> mean weights;  per-dst totals for balancing
    cnt = np.bincount(dst * R + rel, minlength=N * R)
    deg = cnt.reshape(N, R).sum(1)

    # sorted-serpentine: nodes by degree desc, dealt across NBLK blocks
    # alternating direction each round -> near-optimal edge balance.
    order = np.argsort(-deg, kind="stable")
    idx = np.arange(N)
    rnd = idx // NBLK
    pos = idx % NBLK
    blk = np.where(rnd % 2 == 0, pos, NBLK - 1 - pos)
    node_block = np.empty(N, np.int64)
    node_lane = np.empty(N, np.int64)
    node_block[order] = blk
    node_lane[order] = rnd

    node_core = node_block // BPC
    node_pos = (node_block % BPC) * 128 + node_lane      # position in shard
    node_row = node_core * SHARD + node_pos              # row in gather table

    # edge buckets keyed by destination block
    key = node_block[dst]
    bucket_cnt = np.bincount(key, minlength=NBLK)
    T_pad = int(-(-bucket_cnt.max() // 128))

    CAP = T_pad * 128
    order_e = np.argsort(key, kind="stable")
    ks = key[order_e]
    start = np.zeros(NBLK, np.int64)
    start[1:] = np.cumsum(bucket_cnt)[:-1]
    pos_in_bucket = np.arange(E) - start[ks]
    slot = ks * CAP + pos_in_bucket                      # global slot id

    # one int32 per edge slot: row | cv<<17 ; padding -> cv=511 (never matches)
    packed = np.full(NBLK * CAP, np.int32(511 << 17), np.int32)
    wv = np.zeros(NBLK * CAP, np.float32)
    se, de, re_ = src[order_e], dst[order_e], rel[order_e]
    cv = re_ * 128 + node_lane[de]
    packed[slot] = (node_row[se] | (cv << 17)).astype(np.int32)
    wv[slot] = (1.0 / cnt[de * R + re_]).astype(np.float32)

    # reshape to per-core SBUF layouts [128, BPC*T_pad]
    def to_sbuf(a):
        # [NBLK, T_pad, 128] -> per core [128, BPC*T_pad]
        a = a.reshape(N_CORES, BPC, T_pad, 128)
        return np.ascontiguousarray(a.transpose(0, 3, 1, 2).reshape(N_CORES, 128, BPC * T_pad))

    packed_c = to_sbuf(packed)
    wv_c = to_sbuf(wv).astype(BF)

    # host encoder: x = lrelu(W_in . lrelu(enc(feat)) + b_in)   [N, 128] f32
    g = lambda k: np.asarray(inputs[k], np.float32)
    d = _lrelu_np(g("des") @ g("W_des") + g("b_des"))
    t = _lrelu_np(g("tweet") @ g("W_tweet") + g("b_tweet"))
    n = _lrelu_np(g("num_prop") @ g("W_num") + g("b_num"))
    c = _lrelu_np(g("cat_prop") @ g("W_cat") + g("b_cat"))
    x0 = np.concatenate([d, t, n, c], axis=1)
    x = _lrelu_np(x0 @ g("W_in") + g("b_in"))

    # per-feature int8 quantization
    xs = np.maximum(np.abs(x).max(axis=0), 1e-12).astype(np.float32) / 127.0
    xq = np.clip(np.rint(x / xs), -127, 127).astype(np.int8)

    # permute into table order, node-major per core [SHARD, 128]
    row_node = np.full(TROWS, -1, np.int64)
    row_node[node_row] = np.arange(N)
    valid = row_node >= 0
    safe = np.where(valid, row_node, 0)
    Xr = xq[safe]
    Xr[~valid] = 0
    xq_c = np.ascontiguousarray(Xr.reshape(N_CORES, SHARD, 128))

    cfg = dict(N=N, E=E, BPC=BPC, SHARD=SHARD, TROWS=TROWS, T_pad=T_pad)
    per_core = dict(packed=packed_c, wv=wv_c, xq=xq_c)
    asm = dict(node_core=node_core, node_pos=node_pos)
    extra = dict(xs=xs.reshape(128, 1))
    return cfg, per_core, asm, extra


def _weights_inputs(inputs, extra):
    g = lambda k: np.asarray(inputs[k], dtype=np.float32)
    w = {}
    for k, srck in (("Wroot", "W_root"), ("Wo1", "W_o1"), ("Wo2", "W_o2")):
        w[k] = g(srck).astype(BF)
    wrel = g("W_rel")
    w["Wrel0"] = np.ascontiguousarray(wrel[0]).astype(BF)
    w["Wrel1"] = np.ascontiguousarray(wrel[1]).astype(BF)
    w["brg"] = g("b_rgcn").reshape(128, 1)
    w["bo1"] = g("b_o1").reshape(128, 1)
    w["bo2"] = g("b_o2").reshape(2, 1)
    w["xs"] = extra["xs"]
    return w


# ----------------------------------------------------------------------------
# device program
# ----------------------------------------------------------------------------

def _enc_slices(shard):
    out, c = [], 0
    while c < shard:
        w = min(512, shard - c)
        out.append((c, w))
        c += w
    return out


def build_bass(cfg, sim_compat=False):
    BPC, SHARD, TROWS, T_pad = cfg["BPC"], cfg["SHARD"], cfg["TROWS"], cfg["T_pad"]
    NT = BPC * T_pad
    nc = bacc.Bacc("TRN2", target_bir_lowering=False, debug=False,
                   num_devices=N_CORES)

    din = lambda n, s, d: nc.dram_tensor(n, list(s), d, kind="ExternalInput")
    xq = din("xq", (SHARD, 128), I8)
    packed = din("packed", (128, NT), I32)
    wvals = din("wvals", (128, NT), BF16)
    Wroot = din("Wroot", (128, 128), BF16)
    Wrel0, Wrel1 = din("Wrel0", (128, 128), BF16), din("Wrel1", (128, 128), BF16)
    Wo1, Wo2 = din("Wo1", (128, 128), BF16), din("Wo2", (128, 2), BF16)
    brg = din("brg", (128, 1), F32)
    bo1, bo2 = din("bo1", (128, 1), F32), din("bo2", (2, 1), F32)
    xs = din("xs", (128, 1), F32)
    out = nc.dram_tensor("out", [2, SHARD], F32, kind="ExternalOutput")

    groups = [list(range(N_CORES))]
    AG = "AllGather"
    BY = mybir.AluOpType.bypass

    def _lrelu(pool, ps_ap, bias_ap, w, name):
        t = pool.tile([ps_ap.shape[0], w], BF16, name=name)
        if not sim_compat:
            nc.scalar.activation(out=t[:], in_=ps_ap,
                                 func=mybir.ActivationFunctionType.Prelu,
                                 bias=bias_ap, scale=1.0, alpha=ALPHA)
            return t
        zt = pool.tile([ps_ap.shape[0], w], F32, name=name + "_z")
        nc.scalar.activation(out=zt[:], in_=ps_ap,
                             func=mybir.ActivationFunctionType.Identity,
                             bias=bias_ap, scale=1.0)
        rt = pool.tile([ps_ap.shape[0], w], F32, name=name + "_r")
        nc.scalar.activation(out=rt[:], in_=ps_ap,
                             func=mybir.ActivationFunctionType.Relu,
                             bias=bias_ap, scale=1.0)
        t1 = pool.tile([ps_ap.shape[0], w], F32, name=name + "_t1")
        nc.vector.tensor_scalar(out=t1[:], in0=zt[:], scalar1=ALPHA, scalar2=None,
                                op0=mybir.AluOpType.mult)
        t2 = pool.tile([ps_ap.shape[0], w], F32, name=name + "_t2")
        nc.vector.tensor_scalar(out=t2[:], in0=rt[:], scalar1=1.0 - ALPHA, scalar2=None,
                                op0=mybir.AluOpType.mult)
        nc.vector.tensor_tensor(out=t[:], in0=t1[:], in1=t2[:],
                                op=mybir.AluOpType.add)
        return t

    with tile.TileContext(nc) as tc:
        with tc.tile_pool(name="const", bufs=1) as cp, \
             tc.tile_pool(name="dram", bufs=1, space="DRAM") as dp:
            # constants
            c_praw = cp.tile([128, NT], I32); nc.sync.dma_start(c_praw[:], packed[:])
            c_gidx = cp.tile([128, NT], I32)
            nc.vector.tensor_scalar(out=c_gidx[:], in0=c_praw[:],
                                    scalar1=0x1FFFF, scalar2=None,
                                    op0=mybir.AluOpType.bitwise_and)
            c_cvi = cp.tile([128, NT], I32)
            nc.vector.tensor_scalar(out=c_cvi[:], in0=c_praw[:],
                                    scalar1=17, scalar2=None,
                                    op0=mybir.AluOpType.logical_shift_right)
            c_cv = cp.tile([128, NT], F32)
            nc.vector.tensor_copy(out=c_cv[:], in_=c_cvi[:])
            c_wv16 = cp.tile([128, NT], BF16); nc.sync.dma_start(c_wv16[:], wvals[:])
            c_wv = cp.tile([128, NT], F32)
            nc.vector.tensor_copy(out=c_wv[:], in_=c_wv16[:])
            c_ioti = cp.tile([128, 256], I32)
            nc.gpsimd.iota(c_ioti[:], pattern=[[1, 256]], base=0,
                           channel_multiplier=0)
            c_iota = cp.tile([128, 256], F32)
            nc.vector.tensor_copy(out=c_iota[:], in_=c_ioti[:])
            c_Wroot = cp.tile([128, 128], BF16); nc.sync.dma_start(c_Wroot[:], Wroot[:])
            c_Wrel0 = cp.tile([128, 128], BF16); nc.sync.dma_start(c_Wrel0[:], Wrel0[:])
            c_Wrel1 = cp.tile([128, 128], BF16); nc.sync.dma_start(c_Wrel1[:], Wrel1[:])
            c_Wo1 = cp.tile([128, 128], BF16); nc.sync.dma_start(c_Wo1[:], Wo1[:])
            c_Wo2 = cp.tile([128, 2], BF16); nc.sync.dma_start(c_Wo2[:], Wo2[:])
            c_brg = cp.tile([128, 1], F32); nc.sync.dma_start(c_brg[:], brg[:])
            c_bo1 = cp.tile([128, 1], F32); nc.sync.dma_start(c_bo1[:], bo1[:])
            c_bo2 = cp.tile([2, 1], F32); nc.sync.dma_start(c_bo2[:], bo2[:])
            c_xs = cp.tile([128, 1], F32); nc.sync.dma_start(c_xs[:], xs[:])
            ident = cp.tile([128, 128], BF16)
            make_identity(nc, ident[:])

            # DRAM intermediates
            xfm = [dp.tile([128, SHARD], BF16, name=f"xfm{i}") for i in range(3)]
            xnm = [dp.tile([SHARD, 128], BF16, name=f"xnm{i}") for i in range(2)]
            tables = [dp.tile([TROWS, 128], BF16, addr_space="Shared", name=f"table{i}")
                      for i in range(2)]

            # ---------------- ingest: int8 x -> bf16 table + scaled fm ----------
            with tc.tile_pool(name="ing", bufs=4) as ip, \
                 tc.tile_pool(name="ingps", bufs=2, space="PSUM") as ips:
                for k in range(BPC):
                    nm8 = ip.tile([128, 128], I8, name="nm8")
                    nc.sync.dma_start(nm8[:], xq[k * 128:(k + 1) * 128, :])
                    nmb = ip.tile([128, 128], BF16, name="nmb")
                    nc.vector.tensor_copy(out=nmb[:], in_=nm8[:])
                    nc.sync.dma_start(xnm[0][k * 128:(k + 1) * 128, :], nmb[:])
                    ps_t = ips.tile([128, 128], BF16, name="ps_t")
                    nc.tensor.matmul(out=ps_t[:], lhsT=nmb[:], rhs=ident[:],
                                     is_transpose=True, start=True, stop=True)
                    fm = ip.tile([128, 128], BF16, name="fm")
                    nc.vector.tensor_scalar(out=fm[:], in0=ps_t[:],
                                            scalar1=c_xs[:], scalar2=None,
                                            op0=mybir.AluOpType.mult)
                    nc.sync.dma_start(xfm[0][:, k * 128:(k + 1) * 128], fm[:])

            nc.gpsimd.collective_compute(AG, BY, replica_groups=groups,
                                         ins=[xnm[0].opt()], outs=[tables[0].opt()])

            # ---------------- rgcn layers ----------------
            for L in range(2):
                table, xin, xout = tables[L], xfm[L], xfm[L + 1]
                with tc.tile_pool(name=f"gp{L}", bufs=16) as gp, \
                     tc.tile_pool(name=f"sp{L}", bufs=8) as sp, \
                     tc.tile_pool(name=f"up{L}", bufs=2) as up, \
                     tc.tile_pool(name=f"Sps{L}", bufs=4, space="PSUM") as Sps, \
                     tc.tile_pool(name=f"aps{L}", bufs=2, space="PSUM") as aps, \
                     tc.tile_pool(name=f"tps{L}", bufs=2, space="PSUM") as tps:
                    n_units = BPC // 2
                    for u in range(n_units):
                        psS = []
                        for h in range(2):
                            b = u * 2 + h
                            ps = Sps.tile([128, 256], F32, name="psS")
                            psS.append(ps)
                            for t in range(T_pad):
                                T = b * T_pad + t
                                G = gp.tile([128, 128], BF16, name="G")
                                nc.gpsimd.indirect_dma_start(
                                    out=G[:], out_offset=None, in_=table[:],
                                    in_offset=bass.IndirectOffsetOnAxis(
                                        ap=c_gidx[:, T:T + 1], axis=0))
                                sel = sp.tile([128, 256], BF16, name="sel")
                                nc.vector.tensor_scalar(
                                    out=sel[:], in0=c_iota[:],
                                    scalar1=c_cv[:, T:T + 1], scalar2=c_wv[:, T:T + 1],
                                    op0=mybir.AluOpType.is_equal,
                                    op1=mybir.AluOpType.mult)
                                nc.tensor.matmul(out=ps[:], lhsT=G[:], rhs=sel[:],
                                                 start=(t == 0), stop=(t == T_pad - 1))
                        # unit tail: transforms for 2 blocks (256 dst cols)
                        U0 = up.tile([128, 256], BF16, name="U0")
                        U1 = up.tile([128, 256], BF16, name="U1")
                        for h in range(2):
                            if L == 0:
                                # apply int8 dequant scale per feature
                                nc.vector.tensor_scalar(
                                    out=U0[:, h * 128:(h + 1) * 128],
                                    in0=psS[h][:, 0:128], scalar1=c_xs[:],
                                    scalar2=None, op0=mybir.AluOpType.mult)
                                nc.vector.tensor_scalar(
                                    out=U1[:, h * 128:(h + 1) * 128],
                                    in0=psS[h][:, 128:256], scalar1=c_xs[:],
                                    scalar2=None, op0=mybir.AluOpType.mult)
                            else:
                                nc.vector.tensor_copy(
                                    out=U0[:, h * 128:(h + 1) * 128],
                                    in_=psS[h][:, 0:128])
                                nc.vector.tensor_copy(
                                    out=U1[:, h * 128:(h + 1) * 128],
                                    in_=psS[h][:, 128:256])
                        xr = up.tile([128, 256], BF16, name="xr")
                        nc.sync.dma_start(xr[:], xin[:, u * 256:(u + 1) * 256])
                        agg = aps.tile([128, 256], F32, name="agg")
                        nc.tensor.matmul(out=agg[:], lhsT=c_Wroot[:], rhs=xr[:],
                                         start=True, stop=False)
                        nc.tensor.matmul(out=agg[:], lhsT=c_Wrel0[:], rhs=U0[:],
                                         start=False, stop=False)
                        nc.tensor.matmul(out=agg[:], lhsT=c_Wrel1[:], rhs=U1[:],
                                         start=False, stop=True)
                        y = up.tile([128, 256], BF16, name="y")
                        nc.scalar.activation(out=y[:], in_=agg[:],
                                             func=mybir.ActivationFunctionType.Identity,
                                             bias=c_brg[:], scale=1.0)
                        nc.sync.dma_start(xout[:, u * 256:(u + 1) * 256], y[:])
                        if L == 0:
                            for j in range(2):
                                ps_t = tps.tile([128, 128], BF16, name="ps_t2")
                                nc.tensor.matmul(
                                    out=ps_t[:],
                                    lhsT=y[:, j * 128:(j + 1) * 128],
                                    rhs=ident[:], is_transpose=True,
                                    start=True, stop=True)
                                tr_t = up.tile([128, 128], BF16, name="tr2")
                                nc.vector.tensor_copy(out=tr_t[:], in_=ps_t[:])
                                nc.sync.dma_start(
                                    xnm[1][u * 256 + j * 128:u * 256 + (j + 1) * 128, :],
                                    tr_t[:])
                if L == 0:
                    nc.gpsimd.collective_compute(AG, BY, replica_groups=groups,
                                                 ins=[xnm[1].opt()],
                                                 outs=[tables[1].opt()])

            # ---------------- head ----------------
            with tc.tile_pool(name="hd", bufs=3) as hp, \
                 tc.tile_pool(name="hps", bufs=2, space="PSUM") as hps:
                for (c0, w) in _enc_slices(SHARD):
                    xt = hp.tile([128, w], BF16, name="xt")
                    nc.sync.dma_start(xt[:], xfm[2][:, c0:c0 + w])
                    ps_h = hps.tile([128, w], F32, name="ps_h")
                    nc.tensor.matmul(out=ps_h[:], lhsT=c_Wo1[:], rhs=xt[:],
                                     start=True, stop=True)
                    z_t = _lrelu(hp, ps_h[:], c_bo1[:], w, "z_t")
                    ps_o = hps.tile([2, w], F32, name="ps_o")
                    nc.tensor.matmul(out=ps_o[:], lhsT=c_Wo2[:], rhs=z_t[:],
                                     start=True, stop=True)
                    o_t = hp.tile([2, w], F32, name="o_t")
                    nc.scalar.activation(out=o_t[:], in_=ps_o[:],
                                         func=mybir.ActivationFunctionType.Identity,
                                         bias=c_bo2[:], scale=1.0)
                    nc.sync.dma_start(out[:, c0:c0 + w], o_t[:])
    nc.compile()
    return nc


# ----------------------------------------------------------------------------
# cached PJRT runner (jit trace + NEFF compile + device load happen once)
# ----------------------------------------------------------------------------

class _Runner:
    def __init__(self, cfg):
        self.cfg = cfg
        self.nc = build_bass(cfg)
        b2j.install_neuronx_cc_hook()
        nc = self.nc
        partition_name = (nc.partition_id_tensor.name
                          if nc.partition_id_tensor else None)
        in_names, out_names, out_avals = [], [], []
        for alloc in nc.m.functions[0].allocations:
            if not isinstance(alloc, mybir.MemoryLocationSet):
                continue
            name = alloc.memorylocations[0].name
            if alloc.kind == "ExternalInput":
                if name != partition_name:
                    in_names.append(name)
            elif alloc.kind == "ExternalOutput":
                shape = tuple(alloc.tensor_shape)
                dtype = mybir.dt.np(alloc.dtype)
                out_names.append(name)
                out_avals.append(jax.core.ShapedArray(shape, dtype))
        self.in_names = list(in_names)
        self.out_names = out_names
        self.out_avals = out_avals
        n_params = len(in_names)
        n_outs = len(out_avals)
        bind_names = in_names + out_names
        if partition_name is not None:
            bind_names = bind_names + [partition_name]

        def _body(*args):
            operands = list(args)
            if partition_name is not None:
                operands.append(b2j.partition_id_tensor())
            outs = b2j._bass_exec_p.bind(
                *operands,
                out_avals=tuple(out_avals),
                in_names=tuple(bind_names),
                out_names=tuple(out_names),
                lowering_input_output_aliases=(),
                sim_require_finite=True,
                sim_require_nnan=True,
                nc=nc,
            )
            return tuple(outs)

        devices = jax.devices()[:N_CORES]
        mesh = Mesh(np.asarray(devices), ("core",))
        in_specs = (PartitionSpec("core"),) * (n_params + n_outs)
        out_specs = (PartitionSpec("core"),) * n_outs
        self.sharded = jax.jit(
            shard_map(_body, mesh=mesh, in_specs=in_specs, out_specs=out_specs,
                      check_rep=False),
            donate_argnums=tuple(range(n_params, n_params + n_outs)),
            keep_unused=True,
        )

    def __call__(self, maps):
        concat_in = [
            np.concatenate([np.asarray(m[n]) for m in maps], axis=0)
            for n in self.in_names
        ]
        concat_zeros = [
            np.zeros((N_CORES * a.shape[0], *a.shape[1:]), a.dtype)
            for a in self.out_avals
        ]
        outs = self.sharded(*concat_in, *concat_zeros)
        return [
            {name: np.asarray(outs[i]).reshape(N_CORES, *self.out_avals[i].shape)[c]
             for i, name in enumerate(self.out_names)}
            for c in range(N_CORES)
        ]


_RUNNERS = {}


def _get_runner(cfg):
    key = (cfg["N"], cfg["E"], cfg["T_pad"])
    r = _RUNNERS.get(key)
    if r is None:
        r = _Runner(cfg)
        _RUNNERS[key] = r
    return r


# ----------------------------------------------------------------------------
# entry point
# ----------------------------------------------------------------------------

def _in_maps(cfg, per_core, w):
    maps = []
    for c in range(N_CORES):
        m = dict(xq=per_core["xq"][c], packed=per_core["packed"][c],
                 wvals=per_core["wv"][c])
        m.update({k: w[k] for k in ("Wroot", "Wrel0", "Wrel1", "Wo1", "Wo2",
                                    "brg", "bo1", "bo2", "xs")})
        maps.append(m)
    return maps


def _assemble(cfg, asm, core_outs):
    stacked = np.stack([co["out"] for co in core_outs])      # [8, 2, SHARD]
    out = stacked[asm["node_core"], :, asm["node_pos"]]       # [N, 2]
    return np.ascontiguousarray(out.astype(np.float32))


def kernel(**inputs):
    cfg, per_core, asm, extra = _prep(inputs)
    w = _weights_inputs(inputs, extra)
    runner = _get_runner(cfg)
    maps = _in_maps(cfg, per_core, w)
    res = runner(maps)
    return _assemble(cfg, asm, res)


# revision 9
# speedup vs baseline: 10.1118x; 1.1586x over previous
"""BotRGCN on 8 Trainium2 NeuronCores (Bass/Tile).

Strategy (1-D destination-sharded graph partition):
  - Host assigns nodes to 8*BPC blocks of 128 destinations via sorted-serpentine
    balancing on in-degree, so every block holds ~E/(8*BPC) edges; shard s =
    blocks [s*BPC, (s+1)*BPC).  Edges are bucketed per (core, dst-block) and
    padded to a uniform T_pad tiles of 128 edges.
  - The tiny encoder MLP (4.7 GFLOP) runs on host in f32; nodes ship as the
    128-d embedding x quantized int8 with per-feature scales (the wall-clock
    is dominated by the tunneled host->device link, so wire bytes rule).
  - Each core: int8 x -> bf16 (raw ints) -> AllGather into a replicated bf16
    gather table; PE transposes build the feature-major copy with the
    dequant scale applied per partition.  Per dst-block: per-tile
    indirect-DMA row gather + DVE weighted one-hot (tensor_scalar
    is_equal*mult) + scatter-matmul accumulating relation-split sums in
    PSUM (f32); per 2-block unit: W_rel / W_root transform matmuls + bias.
    Layer-1 U copies apply the int8 dequant scale; layer 2 runs true-scale.
    Head = two matmuls + Prelu; per-core output [2, SHARD], host
    inverse-permutes to [N, 2].
  - Edge metadata ships as ONE int32 per edge slot: gather row (17 bits) |
    dst (lane,rel) selector cv (9 bits, 511 = padding sentinel) | mean
    count (6 bits, wv = 1/cnt via DVE reciprocal on device).  Everything
    except the int8 x ships as a single bf16 blob per core (edge words
    bitcast, weights, biases, scales) to minimize per-transfer overhead.
    The compiled executable + jit trace are cached across calls, so each
    call pays only transfer + execute.
"""

import numpy as np
import ml_dtypes

import jax
from jax.sharding import Mesh, PartitionSpec
from jax.experimental.shard_map import shard_map

import concourse.bacc as bacc
import concourse.bass as bass
import concourse.bass2jax as b2j
import concourse.mybir as mybir
import concourse.tile as tile
from concourse.masks import make_identity

F32 = mybir.dt.float32
BF16 = mybir.dt.bfloat16
I32 = mybir.dt.int32
I8 = mybir.dt.int8
BF = ml_dtypes.bfloat16

N_CORES = 8
D = 128
R = 2
ALPHA = 0.01
W_W = 514           # weight section cols: Wroot|Wrel0|Wrel1|Wo1 (4*128) + Wo2 (2)


def _lrelu_np(v):
    return np.where(v > 0, v, np.float32(ALPHA) * v)


def _blob_width(NT, wv_wire):
    return 2 * NT + (NT if wv_wire else 0) + W_W + 8


# ----------------------------------------------------------------------------
# host-side preprocessing (vectorized): graph partition + encoder + quant
# ----------------------------------------------------------------------------

def _prep(inputs):
    src = np.asarray(inputs["edge_index"][0], dtype=np.int64)
    dst = np.asarray(inputs["edge_index"][1], dtype=np.int64)
    rel = np.asarray(inputs["edge_type"], dtype=np.int64)
    N = int(np.asarray(inputs["des"]).shape[0])
    E = src.shape[0]

    BPC = (-(-N // N_CORES) + 127) // 128  # ceil(ceil(N/8)/128)
    SHARD = BPC * 128
    NBLK = N_CORES * BPC
    TROWS = N_CORES * SHARD
    assert N <= NBLK * 128
    assert TROWS <= (1 << 17), "gather row must fit 17 bits"

    # per-(dst,rel) counts -> mean weights;  per-dst totals for balancing
    cnt = np.bincount(dst * R + rel, minlength=N * R)
    deg = cnt.reshape(N, R).sum(1)

    # sorted-serpentine: nodes by degree desc, dealt across NBLK blocks
    # alternating direction each round -> near-optimal edge balance.
    order = np.argsort(-deg, kind="stable")
    idx = np.arange(N)
    rnd = idx // NBLK
    pos = idx % NBLK
    blk = np.where(rnd % 2 == 0, pos, NBLK - 1 - pos)
    node_block = np.empty(N, np.int64)
    node_lane = np.empty(N, np.int64)
    node_block[order] = blk
    node_lane[order] = rnd

    node_core = node_block // BPC
    node_pos = (node_block % BPC) * 128 + node_lane      # position in shard
    node_row = node_core * SHARD + node_pos              # row in gather table

    # edge buckets keyed by destination block
    key = node_block[dst]
    bucket_cnt = np.bincount(key, minlength=NBLK)
    T_pad = int(-(-bucket_cnt.max() // 128))

    CAP = T_pad * 128
    order_e = np.argsort(key, kind="stable")
    ks = key[order_e]
    start = np.zeros(NBLK, np.int64)
    start[1:] = np.cumsum(bucket_cnt)[:-1]
    pos_in_bucket = np.arange(E) - start[ks]
    slot = ks * CAP + pos_in_bucket                      # global slot id

    # one int32 per edge slot: row | cv<<17 | cnt<<26
    # padding: cv=511 (never matches the 0..255 iota), cnt=1 (finite 1/cnt)
    se, de, re_ = src[order_e], dst[order_e], rel[order_e]
    cv = re_ * 128 + node_lane[de]
    cntv = cnt[de * R + re_]
    wv_wire = bool(cntv.max() > 63)
    packed = np.full(NBLK * CAP, np.int32((511 << 17) | (1 << 26)), np.int32)
    if wv_wire:
        packed[slot] = (node_row[se] | (cv << 17) | (1 << 26)).astype(np.int32)
    else:
        packed[slot] = (node_row[se] | (cv << 17) | (cntv << 26)).astype(np.int32)

    # reshape to per-core SBUF layouts [128, BPC*T_pad]
    def to_sbuf(a):
        # [NBLK, T_pad, 128] -> per core [128, BPC*T_pad]
        a = a.reshape(N_CORES, BPC, T_pad, 128)
        return np.ascontiguousarray(a.transpose(0, 3, 1, 2).reshape(N_CORES, 128, BPC * T_pad))

    packed_c = to_sbuf(packed)
    NT = BPC * T_pad

    # host encoder: x = lrelu(W_in . lrelu(enc(feat)) + b_in)   [N, 128] f32
    g = lambda k: np.asarray(inputs[k], np.float32)
    d = _lrelu_np(g("des") @ g("W_des") + g("b_des"))
    t = _lrelu_np(g("tweet") @ g("W_tweet") + g("b_tweet"))
    n = _lrelu_np(g("num_prop") @ g("W_num") + g("b_num"))
    c = _lrelu_np(g("cat_prop") @ g("W_cat") + g("b_cat"))
    x0 = np.concatenate([d, t, n, c], axis=1)
    x = _lrelu_np(x0 @ g("W_in") + g("b_in"))

    # per-feature int8 quantization
    xs = np.maximum(np.abs(x).max(axis=0), 1e-12).astype(np.float32) / 127.0
    xq = np.clip(np.rint(x / xs), -127, 127).astype(np.int8)

    # permute into table order, node-major per core [SHARD, 128]
    row_node = np.full(TROWS, -1, np.int64)
    row_node[node_row] = np.arange(N)
    valid = row_node >= 0
    safe = np.where(valid, row_node, 0)
    Xr = xq[safe]
    Xr[~valid] = 0
    xq_c = np.ascontiguousarray(Xr.reshape(N_CORES, SHARD, 128))

    # the bf16 blob: packed (bitcast) | [wv] | weights | f32 biases+scales
    W_BIG = _blob_width(NT, wv_wire)
    oW = W_BIG - W_W - 8
    oF = W_BIG - 8
    big = np.zeros((N_CORES, 128, W_BIG), BF)
    big[:, :, 0:2 * NT] = packed_c.view(BF)
    if wv_wire:
        wv = np.zeros(NBLK * CAP, np.float32)
        wv[slot] = (1.0 / cntv).astype(np.float32)
        big[:, :, 2 * NT:3 * NT] = to_sbuf(wv).astype(BF)
    wrel = g("W_rel")
    Wsec = np.concatenate(
        [g("W_root"), wrel[0], wrel[1], g("W_o1"), g("W_o2")], axis=1).astype(BF)
    big[:, :, oW:oW + W_W] = Wsec
    f32sec = np.zeros((128, 4), np.float32)
    f32sec[:, 0] = g("b_rgcn")
    f32sec[:, 1] = g("b_o1")
    f32sec[:, 2] = xs
    f32sec[0:2, 3] = g("b_o2")
    big[:, :, oF:oF + 8] = f32sec.view(BF)

    cfg = dict(N=N, E=E, BPC=BPC, SHARD=SHARD, TROWS=TROWS, T_pad=T_pad,
               wv_wire=wv_wire)
    per_core = dict(big=big, xq=xq_c)
    asm = dict(node_core=node_core, node_pos=node_pos)
    return cfg, per_core, asm


# ----------------------------------------------------------------------------
# device program
# ----------------------------------------------------------------------------

def _enc_slices(shard):
    out, c = [], 0
    while c < shard:
        w = min(512, shard - c)
        out.append((c, w))
        c += w
    return out


def build_bass(cfg, sim_compat=False):
    BPC, SHARD, TROWS, T_pad = cfg["BPC"], cfg["SHARD"], cfg["TROWS"], cfg["T_pad"]
    wv_wire = cfg["wv_wire"]
    NT = BPC * T_pad
    W_BIG = _blob_width(NT, wv_wire)
    oW = W_BIG - W_W - 8
    oF = W_BIG - 8
    nc = bacc.Bacc("TRN2", target_bir_lowering=False, debug=False,
                   num_devices=N_CORES)

    blob = nc.dram_tensor("blob", [128, W_BIG], BF16, kind="ExternalInput")
    xq = nc.dram_tensor("xq", [SHARD, 128], I8, kind="ExternalInput")
    out = nc.dram_tensor("out", [2, SHARD], F32, kind="ExternalOutput")

    groups = [list(range(N_CORES))]
    AG = "AllGather"
    BY = mybir.AluOpType.bypass

    def _lrelu(pool, ps_ap, bias_ap, w, name):
        t = pool.tile([ps_ap.shape[0], w], BF16, name=name)
        if not sim_compat:
            nc.scalar.activation(out=t[:], in_=ps_ap,
                                 func=mybir.ActivationFunctionType.Prelu,
                                 bias=bias_ap, scale=1.0, alpha=ALPHA)
            return t
        zt = pool.tile([ps_ap.shape[0], w], F32, name=name + "_z")
        nc.scalar.activation(out=zt[:], in_=ps_ap,
                             func=mybir.ActivationFunctionType.Identity,
                             bias=bias_ap, scale=1.0)
        rt = pool.tile([ps_ap.shape[0], w], F32, name=name + "_r")
        nc.scalar.activation(out=rt[:], in_=ps_ap,
                             func=mybir.ActivationFunctionType.Relu,
                             bias=bias_ap, scale=1.0)
        t1 = pool.tile([ps_ap.shape[0], w], F32, name=name + "_t1")
        nc.vector.tensor_scalar(out=t1[:], in0=zt[:], scalar1=ALPHA, scalar2=None,
                                op0=mybir.AluOpType.mult)
        t2 = pool.tile([ps_ap.shape[0], w], F32, name=name + "_t2")
        nc.vector.tensor_scalar(out=t2[:], in0=rt[:], scalar1=1.0 - ALPHA, scalar2=None,
                                op0=mybir.AluOpType.mult)
        nc.vector.tensor_tensor(out=t[:], in0=t1[:], in1=t2[:],
                                op=mybir.AluOpType.add)
        return t

    with tile.TileContext(nc) as tc:
        with tc.tile_pool(name="const", bufs=1) as cp, \
             tc.tile_pool(name="dram", bufs=1, space="DRAM") as dp:
            # unpack the blob
            c_praw = cp.tile([128, NT], I32)
            nc.sync.dma_start(c_praw[:], blob[:, 0:2 * NT].bitcast(I32))
            c_gidx = cp.tile([128, NT], I32)
            nc.vector.tensor_scalar(out=c_gidx[:], in0=c_praw[:],
                                    scalar1=0x1FFFF, scalar2=None,
                                    op0=mybir.AluOpType.bitwise_and)
            c_cvi = cp.tile([128, NT], I32)
            nc.vector.tensor_scalar(out=c_cvi[:], in0=c_praw[:],
                                    scalar1=17, scalar2=0x1FF,
                                    op0=mybir.AluOpType.logical_shift_right,
                                    op1=mybir.AluOpType.bitwise_and)
            c_cv = cp.tile([128, NT], F32)
            nc.vector.tensor_copy(out=c_cv[:], in_=c_cvi[:])
            c_wv = cp.tile([128, NT], F32)
            if wv_wire:
                c_wv16 = cp.tile([128, NT], BF16)
                nc.sync.dma_start(c_wv16[:], blob[:, 2 * NT:3 * NT])
                nc.vector.tensor_copy(out=c_wv[:], in_=c_wv16[:])
            else:
                c_cnti = cp.tile([128, NT], I32)
                nc.vector.tensor_scalar(out=c_cnti[:], in0=c_praw[:],
                                        scalar1=26, scalar2=None,
                                        op0=mybir.AluOpType.logical_shift_right)
                c_cntf = cp.tile([128, NT], F32)
                nc.vector.tensor_copy(out=c_cntf[:], in_=c_cnti[:])
                nc.vector.reciprocal(out=c_wv[:], in_=c_cntf[:])
            c_ioti = cp.tile([128, 256], I32)
            nc.gpsimd.iota(c_ioti[:], pattern=[[1, 256]], base=0,
                           channel_multiplier=0)
            c_iota = cp.tile([128, 256], F32)
            nc.vector.tensor_copy(out=c_iota[:], in_=c_ioti[:])
            c_W = cp.tile([128, W_W], BF16)
            nc.sync.dma_start(c_W[:], blob[:, oW:oW + W_W])
            c_Wroot = c_W[:, 0:128]
            c_Wrel0 = c_W[:, 128:256]
            c_Wrel1 = c_W[:, 256:384]
            c_Wo1 = c_W[:, 384:512]
            c_Wo2 = c_W[:, 512:514]
            c_f32 = cp.tile([128, 4], F32)
            nc.sync.dma_start(c_f32[:], blob[:, oF:oF + 8].bitcast(F32))
            c_brg = c_f32[:, 0:1]
            c_bo1 = c_f32[:, 1:2]
            c_xs = c_f32[:, 2:3]
            c_bo2 = c_f32[0:2, 3:4]
            ident = cp.tile([128, 128], BF16)
            make_identity(nc, ident[:])

            # DRAM intermediates
            xfm = [dp.tile([128, SHARD], BF16, name=f"xfm{i}") for i in range(3)]
            xnm = [dp.tile([SHARD, 128], BF16, name=f"xnm{i}") for i in range(2)]
            tables = [dp.tile([TROWS, 128], BF16, addr_space="Shared", name=f"table{i}")
                      for i in range(2)]

            # ---------------- ingest: int8 x -> bf16 table + scaled fm ----------
            with tc.tile_pool(name="ing", bufs=4) as ip, \
                 tc.tile_pool(name="ingps", bufs=2, space="PSUM") as ips:
                for k in range(BPC):
                    nm8 = ip.tile([128, 128], I8, name="nm8")
                    nc.sync.dma_start(nm8[:], xq[k * 128:(k + 1) * 128, :])
                    nmb = ip.tile([128, 128], BF16, name="nmb")
                    nc.vector.tensor_copy(out=nmb[:], in_=nm8[:])
                    nc.sync.dma_start(xnm[0][k * 128:(k + 1) * 128, :], nmb[:])
                    ps_t = ips.tile([128, 128], BF16, name="ps_t")
                    nc.tensor.matmul(out=ps_t[:], lhsT=nmb[:], rhs=ident[:],
                                     is_transpose=True, start=True, stop=True)
                    fm = ip.tile([128, 128], BF16, name="fm")
                    nc.vector.tensor_scalar(out=fm[:], in0=ps_t[:],
                                            scalar1=c_xs, scalar2=None,
                                            op0=mybir.AluOpType.mult)
                    nc.sync.dma_start(xfm[0][:, k * 128:(k + 1) * 128], fm[:])

            nc.gpsimd.collective_compute(AG, BY, replica_groups=groups,
                                         ins=[xnm[0].opt()], outs=[tables[0].opt()])

            # ---------------- rgcn layers ----------------
            for L in range(2):
                table, xin, xout = tables[L], xfm[L], xfm[L + 1]
                with tc.tile_pool(name=f"gp{L}", bufs=16) as gp, \
                     tc.tile_pool(name=f"sp{L}", bufs=8) as sp, \
                     tc.tile_pool(name=f"up{L}", bufs=2) as up, \
                     tc.tile_pool(name=f"Sps{L}", bufs=4, space="PSUM") as Sps, \
                     tc.tile_pool(name=f"aps{L}", bufs=2, space="PSUM") as aps, \
                     tc.tile_pool(name=f"tps{L}", bufs=2, space="PSUM") as tps:
                    n_units = BPC // 2
                    for u in range(n_units):
                        psS = []
                        for h in range(2):
                            b = u * 2 + h
                            ps = Sps.tile([128, 256], F32, name="psS")
                            psS.append(ps)
                            for t in range(T_pad):
                                T = b * T_pad + t
                                G = gp.tile([128, 128], BF16, name="G")
                                nc.gpsimd.indirect_dma_start(
                                    out=G[:], out_offset=None, in_=table[:],
                                    in_offset=bass.IndirectOffsetOnAxis(
                                        ap=c_gidx[:, T:T + 1], axis=0))
                                sel = sp.tile([128, 256], BF16, name="sel")
                                nc.vector.tensor_scalar(
                                    out=sel[:], in0=c_iota[:],
                                    scalar1=c_cv[:, T:T + 1], scalar2=c_wv[:, T:T + 1],
                                    op0=mybir.AluOpType.is_equal,
                                    op1=mybir.AluOpType.mult)
                                nc.tensor.matmul(out=ps[:], lhsT=G[:], rhs=sel[:],
                                                 start=(t == 0), stop=(t == T_pad - 1))
                        # unit tail: transforms for 2 blocks (256 dst cols)
                        U0 = up.tile([128, 256], BF16, name="U0")
                        U1 = up.tile([128, 256], BF16, name="U1")
                        for h in range(2):
                            if L == 0:
                                # apply int8 dequant scale per feature
                                nc.vector.tensor_scalar(
                                    out=U0[:, h * 128:(h + 1) * 128],
                                    in0=psS[h][:, 0:128], scalar1=c_xs,
                                    scalar2=None, op0=mybir.AluOpType.mult)
                                nc.vector.tensor_scalar(
                                    out=U1[:, h * 128:(h + 1) * 128],
                                    in0=psS[h][:, 128:256], scalar1=c_xs,
                                    scalar2=None, op0=mybir.AluOpType.mult)
                            else:
                                nc.vector.tensor_copy(
                                    out=U0[:, h * 128:(h + 1) * 128],
                                    in_=psS[h][:, 0:128])
                                nc.vector.tensor_copy(
                                    out=U1[:, h * 128:(h + 1) * 128],
                                    in_=psS[h][:, 128:256])
                        xr = up.tile([128, 256], BF16, name="xr")
                        nc.sync.dma_start(xr[:], xin[:, u * 256:(u + 1) * 256])
                        agg = aps.tile([128, 256], F32, name="agg")
                        nc.tensor.matmul(out=agg[:], lhsT=c_Wroot, rhs=xr[:],
                                         start=True, stop=False)
                        nc.tensor.matmul(out=agg[:], lhsT=c_Wrel0, rhs=U0[:],
                                         start=False, stop=False)
                        nc.tensor.matmul(out=agg[:], lhsT=c_Wrel1, rhs=U1[:],
                                         start=False, stop=True)
                        y = up.tile([128, 256], BF16, name="y")
                        nc.scalar.activation(out=y[:], in_=agg[:],
                                             func=mybir.ActivationFunctionType.Identity,
                                             bias=c_brg, scale=1.0)
                        nc.sync.dma_start(xout[:, u * 256:(u + 1) * 256], y[:])
                        if L == 0:
                            for j in range(2):
                                ps_t = tps.tile([128, 128], BF16, name="ps_t2")
                                nc.tensor.matmul(
                                    out=ps_t[:],
                                    lhsT=y[:, j * 128:(j + 1) * 128],
                                    rhs=ident[:], is_transpose=True,
                                    start=True, stop=True)
                                tr_t = up.tile([128, 128], BF16, name="tr2")
                                nc.vector.tensor_copy(out=tr_t[:], in_=ps_t[:])
                                nc.sync.dma_start(
                                    xnm[1][u * 256 + j * 128:u * 256 + (j + 1) * 128, :],
                                    tr_t[:])
                if L == 0:
                    nc.gpsimd.collective_compute(AG, BY, replica_groups=groups,
                                                 ins=[xnm[1].opt()],
                                                 outs=[tables[1].opt()])

            # ---------------- head ----------------
            with tc.tile_pool(name="hd", bufs=3) as hp, \
                 tc.tile_pool(name="hps", bufs=2, space="PSUM") as hps:
                for (c0, w) in _enc_slices(SHARD):
                    xt = hp.tile([128, w], BF16, name="xt")
                    nc.sync.dma_start(xt[:], xfm[2][:, c0:c0 + w])
                    ps_h = hps.tile([128, w], F32, name="ps_h")
                    nc.tensor.matmul(out=ps_h[:], lhsT=c_Wo1, rhs=xt[:],
                                     start=True, stop=True)
                    z_t = _lrelu(hp, ps_h[:], c_bo1, w, "z_t")
                    ps_o = hps.tile([2, w], F32, name="ps_o")
                    nc.tensor.matmul(out=ps_o[:], lhsT=c_Wo2, rhs=z_t[:],
                                     start=True, stop=True)
                    o_t = hp.tile([2, w], F32, name="o_t")
                    nc.scalar.activation(out=o_t[:], in_=ps_o[:],
                                         func=mybir.ActivationFunctionType.Identity,
                                         bias=c_bo2, scale=1.0)
                    nc.sync.dma_start(out[:, c0:c0 + w], o_t[:])
    nc.compile()
    return nc


# ----------------------------------------------------------------------------
# cached PJRT runner (jit trace + NEFF compile + device load happen once)
# ----------------------------------------------------------------------------

class _Runner:
    def __init__(self, cfg):
        self.cfg = cfg
        self.nc = build_bass(cfg)
        b2j.install_neuronx_cc_hook()
        nc = self.nc
        partition_name = (nc.partition_id_tensor.name
                          if nc.partition_id_tensor else None)
        in_names, out_names, out_avals = [], [], []
        for alloc in nc.m.functions[0].allocations:
            if not isinstance(alloc, mybir.MemoryLocationSet):
                continue
            name = alloc.memorylocations[0].name
            if alloc.kind == "ExternalInput":
                if name != partition_name:
                    in_names.append(name)
            elif alloc.kind == "ExternalOutput":
                shape = tuple(alloc.tensor_shape)
                dtype = mybir.dt.np(alloc.dtype)
                out_names.append(name)
                out_avals.append(jax.core.ShapedArray(shape, dtype))
        self.in_names = list(in_names)
        self.out_names = out_names
        self.out_avals = out_avals
        n_params = len(in_names)
        n_outs = len(out_avals)
        bind_names = in_names + out_names
        if partition_name is not None:
            bind_names = bind_names + [partition_name]

        def _body(*args):
            operands = list(args)
            if partition_name is not None:
                operands.append(b2j.partition_id_tensor())
            outs = b2j._bass_exec_p.bind(
                *operands,
                out_avals=tuple(out_avals),
                in_names=tuple(bind_names),
                out_names=tuple(out_names),
                lowering_input_output_aliases=(),
                sim_require_finite=True,
                sim_require_nnan=True,
                nc=nc,
            )
            return tuple(outs)

        devices = jax.devices()[:N_CORES]
        mesh = Mesh(np.asarray(devices), ("core",))
        in_specs = (PartitionSpec("core"),) * (n_params + n_outs)
        out_specs = (PartitionSpec("core"),) * n_outs
        self.sharded = jax.jit(
            shard_map(_body, mesh=mesh, in_specs=in_specs, out_specs=out_specs,
                      check_rep=False),
            donate_argnums=tuple(range(n_params, n_params + n_outs)),
            keep_unused=True,
        )

    def __call__(self, maps):
        concat_in = [
            np.concatenate([np.asarray(m[n]) for m in maps], axis=0)
            for n in self.in_names
        ]
        concat_zeros = [
            np.zeros((N_CORES * a.shape[0], *a.shape[1:]), a.dtype)
            for a in self.out_avals
        ]
        outs = self.sharded(*concat_in, *concat_zeros)
        return [
            {name: np.asarray(outs[i]).reshape(N_CORES, *self.out_avals[i].shape)[c]
             for i, name in enumerate(self.out_names)}
            for c in range(N_CORES)
        ]


_RUNNERS = {}


def _get_runner(cfg):
    key = (cfg["N"], cfg["E"], cfg["T_pad"], cfg["wv_wire"])
    r = _RUNNERS.get(key)
    if r is None:
        r = _Runner(cfg)
        _RUNNERS[key] = r
    return r


# ----------------------------------------------------------------------------
# entry point
# ----------------------------------------------------------------------------

def _in_maps(cfg, per_core):
    return [dict(blob=per_core["big"][c], xq=per_core["xq"][c])
            for c in range(N_CORES)]


def _assemble(cfg, asm, core_outs):
    stacked = np.stack([co["out"] for co in core_outs])      # [8, 2, SHARD]
    out = stacked[asm["node_core"], :, asm["node_pos"]]       # [N, 2]
    return np.ascontiguousarray(out.astype(np.float32))


def kernel(**inputs):
    cfg, per_core, asm = _prep(inputs)
    runner = _get_runner(cfg)
    maps = _in_maps(cfg, per_core)
    res = runner(maps)
    return _assemble(cfg, asm, res)


# revision 14
# speedup vs baseline: 12.4016x; 1.2264x over previous
"""BotRGCN on 8 Trainium2 NeuronCores (Bass/Tile).

Strategy (1-D destination-sharded graph partition):
  - Host assigns nodes to 8*BPC blocks of 128 destinations via sorted-serpentine
    balancing on in-degree, so every block holds ~E/(8*BPC) edges; shard s =
    blocks [s*BPC, (s+1)*BPC).  Edges are bucketed per (core, dst-block) and
    padded to a uniform T_pad tiles of 128 edges.
  - The tiny encoder MLP (4.7 GFLOP) runs on host in f32; nodes ship as the
    128-d embedding x quantized int8 with per-feature scales (the wall-clock
    is dominated by the tunneled host->device link, so wire bytes rule).
  - Each core: int8 x -> bf16 (raw ints) -> AllGather into a replicated bf16
    gather table; PE transposes build the feature-major copy with the
    dequant scale applied per partition.  Per dst-block: per-tile
    indirect-DMA row gather + DVE weighted one-hot (tensor_scalar
    is_equal*mult) + scatter-matmul accumulating relation-split sums in
    PSUM (f32); per 2-block unit: W_rel / W_root transform matmuls + bias.
    Layer-1 U copies apply the int8 dequant scale; layer 2 runs true-scale.
    Head = two matmuls + Prelu; per-core output [2, SHARD], host
    inverse-permutes to [N, 2].
  - Edge metadata ships as ONE int32 per edge slot: gather row (17 bits) |
    dst (lane,rel) selector cv (9 bits, 511 = padding sentinel) | mean
    count (6 bits, wv = 1/cnt via DVE reciprocal on device).  Everything
    except the int8 x ships as a single bf16 blob per core (edge words
    bitcast, weights, biases, scales) to minimize per-transfer overhead.
    The compiled executable + jit trace are cached across calls, so each
    call pays only transfer + execute.
"""

import numpy as np
import ml_dtypes

import jax
from jax.sharding import Mesh, PartitionSpec
from jax.experimental.shard_map import shard_map

import concourse.bacc as bacc
import concourse.bass as bass
import concourse.bass2jax as b2j
import concourse.mybir as mybir
import concourse.tile as tile
from concourse.masks import make_identity

F32 = mybir.dt.float32
BF16 = mybir.dt.bfloat16
I32 = mybir.dt.int32
I8 = mybir.dt.int8
BF = ml_dtypes.bfloat16

N_CORES = 8
D = 128
R = 2
ALPHA = 0.01
W_W = 514           # weight section cols: Wroot|Wrel0|Wrel1|Wo1 (4*128) + Wo2 (2)


def _lrelu_np(v):
    return np.where(v > 0, v, np.float32(ALPHA) * v)


def _blob_width(NT, wv_wire):
    return 2 * NT + (NT if wv_wire else 0) + W_W + 8


# ----------------------------------------------------------------------------
# host-side preprocessing (vectorized): graph partition + encoder + quant
# ----------------------------------------------------------------------------

def _prep(inputs):
    src = np.asarray(inputs["edge_index"][0], dtype=np.int64)
    dst = np.asarray(inputs["edge_index"][1], dtype=np.int64)
    rel = np.asarray(inputs["edge_type"], dtype=np.int64)
    N = int(np.asarray(inputs["des"]).shape[0])
    E = src.shape[0]

    BPC = (-(-N // N_CORES) + 127) // 128  # ceil(ceil(N/8)/128)
    SHARD = BPC * 128
    NBLK = N_CORES * BPC
    TROWS = N_CORES * SHARD
    assert N <= NBLK * 128
    assert TROWS <= (1 << 17), "gather row must fit 17 bits"

    # per-(dst,rel) counts -> mean weights;  per-dst totals for balancing
    cnt = np.bincount(dst * R + rel, minlength=N * R)
    deg = cnt.reshape(N, R).sum(1)

    # sorted-serpentine: nodes by degree desc, dealt across NBLK blocks
    # alternating direction each round -> near-optimal edge balance.
    order = np.argsort(-deg, kind="stable")
    idx = np.arange(N)
    rnd = idx // NBLK
    pos = idx % NBLK
    blk = np.where(rnd % 2 == 0, pos, NBLK - 1 - pos)
    node_block = np.empty(N, np.int64)
    node_lane = np.empty(N, np.int64)
    node_block[order] = blk
    node_lane[order] = rnd

    node_core = node_block // BPC
    node_pos = (node_block % BPC) * 128 + node_lane      # position in shard
    node_row = node_core * SHARD + node_pos              # row in gather table

    # edge buckets keyed by destination block
    key = node_block[dst]
    bucket_cnt = np.bincount(key, minlength=NBLK)
    T_pad = int(-(-bucket_cnt.max() // 128))

    CAP = T_pad * 128
    order_e = np.argsort(key, kind="stable")
    ks = key[order_e]
    start = np.zeros(NBLK, np.int64)
    start[1:] = np.cumsum(bucket_cnt)[:-1]
    pos_in_bucket = np.arange(E) - start[ks]
    slot = ks * CAP + pos_in_bucket                      # global slot id

    # one int32 per edge slot: row | cv<<17 | cnt<<26
    # padding: cv=511 (never matches the 0..255 iota), cnt=1 (finite 1/cnt)
    se, de, re_ = src[order_e], dst[order_e], rel[order_e]
    cv = re_ * 128 + node_lane[de]
    cntv = cnt[de * R + re_]
    wv_wire = bool(cntv.max() > 63)
    packed = np.full(NBLK * CAP, np.int32((511 << 17) | (1 << 26)), np.int32)
    if wv_wire:
        packed[slot] = (node_row[se] | (cv << 17) | (1 << 26)).astype(np.int32)
    else:
        packed[slot] = (node_row[se] | (cv << 17) | (cntv << 26)).astype(np.int32)

    # reshape to per-core SBUF layouts [128, BPC*T_pad]
    def to_sbuf(a):
        # [NBLK, T_pad, 128] -> per core [128, BPC*T_pad]
        a = a.reshape(N_CORES, BPC, T_pad, 128)
        return np.ascontiguousarray(a.transpose(0, 3, 1, 2).reshape(N_CORES, 128, BPC * T_pad))

    packed_c = to_sbuf(packed)
    NT = BPC * T_pad

    # host encoder: x = lrelu(W_in . lrelu(enc(feat)) + b_in)   [N, 128] f32
    g = lambda k: np.asarray(inputs[k], np.float32)
    d = _lrelu_np(g("des") @ g("W_des") + g("b_des"))
    t = _lrelu_np(g("tweet") @ g("W_tweet") + g("b_tweet"))
    n = _lrelu_np(g("num_prop") @ g("W_num") + g("b_num"))
    c = _lrelu_np(g("cat_prop") @ g("W_cat") + g("b_cat"))
    x0 = np.concatenate([d, t, n, c], axis=1)
    x = _lrelu_np(x0 @ g("W_in") + g("b_in"))

    # per-feature int8 quantization
    xs = np.maximum(np.abs(x).max(axis=0), 1e-12).astype(np.float32) / 127.0
    xq = np.clip(np.rint(x / xs), -127, 127).astype(np.int8)

    # permute into table order, node-major per core [SHARD, 128]
    row_node = np.full(TROWS, -1, np.int64)
    row_node[node_row] = np.arange(N)
    valid = row_node >= 0
    safe = np.where(valid, row_node, 0)
    Xr = xq[safe]
    Xr[~valid] = 0
    xq_c = np.ascontiguousarray(Xr.reshape(N_CORES, SHARD, 128))

    # the bf16 blob: packed (bitcast) | [wv] | weights | f32 biases+scales
    W_BIG = _blob_width(NT, wv_wire)
    oW = W_BIG - W_W - 8
    oF = W_BIG - 8
    big = np.zeros((N_CORES, 128, W_BIG), BF)
    big[:, :, 0:2 * NT] = packed_c.view(BF)
    if wv_wire:
        wv = np.zeros(NBLK * CAP, np.float32)
        wv[slot] = (1.0 / cntv).astype(np.float32)
        big[:, :, 2 * NT:3 * NT] = to_sbuf(wv).astype(BF)
    wrel = g("W_rel")
    Wsec = np.concatenate(
        [g("W_root"), wrel[0], wrel[1], g("W_o1"), g("W_o2")], axis=1).astype(BF)
    big[:, :, oW:oW + W_W] = Wsec
    f32sec = np.zeros((128, 4), np.float32)
    f32sec[:, 0] = g("b_rgcn")
    f32sec[:, 1] = g("b_o1")
    f32sec[:, 2] = xs
    f32sec[0:2, 3] = g("b_o2")
    big[:, :, oF:oF + 8] = f32sec.view(BF)

    cfg = dict(N=N, E=E, BPC=BPC, SHARD=SHARD, TROWS=TROWS, T_pad=T_pad,
               wv_wire=wv_wire)
    per_core = dict(big=big, xq=xq_c)
    asm = dict(node_core=node_core, node_pos=node_pos)
    return cfg, per_core, asm


# ----------------------------------------------------------------------------
# device program
# ----------------------------------------------------------------------------

def _enc_slices(shard):
    out, c = [], 0
    while c < shard:
        w = min(512, shard - c)
        out.append((c, w))
        c += w
    return out


def build_bass(cfg, sim_compat=False):
    BPC, SHARD, TROWS, T_pad = cfg["BPC"], cfg["SHARD"], cfg["TROWS"], cfg["T_pad"]
    wv_wire = cfg["wv_wire"]
    NT = BPC * T_pad
    W_BIG = _blob_width(NT, wv_wire)
    oW = W_BIG - W_W - 8
    oF = W_BIG - 8
    nc = bacc.Bacc("TRN2", target_bir_lowering=False, debug=False,
                   num_devices=N_CORES)

    blob = nc.dram_tensor("blob", [128, W_BIG], BF16, kind="ExternalInput")
    xq = nc.dram_tensor("xq", [SHARD, 128], I8, kind="ExternalInput")
    out = nc.dram_tensor("out", [2, SHARD], BF16, kind="ExternalOutput")

    groups = [list(range(N_CORES))]
    AG = "AllGather"
    BY = mybir.AluOpType.bypass

    def _lrelu(pool, ps_ap, bias_ap, w, name):
        t = pool.tile([ps_ap.shape[0], w], BF16, name=name)
        if not sim_compat:
            nc.scalar.activation(out=t[:], in_=ps_ap,
                                 func=mybir.ActivationFunctionType.Prelu,
                                 bias=bias_ap, scale=1.0, alpha=ALPHA)
            return t
        zt = pool.tile([ps_ap.shape[0], w], F32, name=name + "_z")
        nc.scalar.activation(out=zt[:], in_=ps_ap,
                             func=mybir.ActivationFunctionType.Identity,
                             bias=bias_ap, scale=1.0)
        rt = pool.tile([ps_ap.shape[0], w], F32, name=name + "_r")
        nc.scalar.activation(out=rt[:], in_=ps_ap,
                             func=mybir.ActivationFunctionType.Relu,
                             bias=bias_ap, scale=1.0)
        t1 = pool.tile([ps_ap.shape[0], w], F32, name=name + "_t1")
        nc.vector.tensor_scalar(out=t1[:], in0=zt[:], scalar1=ALPHA, scalar2=None,
                                op0=mybir.AluOpType.mult)
        t2 = pool.tile([ps_ap.shape[0], w], F32, name=name + "_t2")
        nc.vector.tensor_scalar(out=t2[:], in0=rt[:], scalar1=1.0 - ALPHA, scalar2=None,
                                op0=mybir.AluOpType.mult)
        nc.vector.tensor_tensor(out=t[:], in0=t1[:], in1=t2[:],
                                op=mybir.AluOpType.add)
        return t

    with tile.TileContext(nc) as tc:
        with tc.tile_pool(name="const", bufs=1) as cp, \
             tc.tile_pool(name="dram", bufs=1, space="DRAM") as dp:
            # unpack the blob
            c_praw = cp.tile([128, NT], I32)
            nc.sync.dma_start(c_praw[:], blob[:, 0:2 * NT].bitcast(I32))
            c_gidx = cp.tile([128, NT], I32)
            nc.vector.tensor_scalar(out=c_gidx[:], in0=c_praw[:],
                                    scalar1=0x1FFFF, scalar2=None,
                                    op0=mybir.AluOpType.bitwise_and)
            c_cvi = cp.tile([128, NT], I32)
            nc.vector.tensor_scalar(out=c_cvi[:], in0=c_praw[:],
                                    scalar1=17, scalar2=0x1FF,
                                    op0=mybir.AluOpType.logical_shift_right,
                                    op1=mybir.AluOpType.bitwise_and)
            c_cv = cp.tile([128, NT], F32)
            nc.vector.tensor_copy(out=c_cv[:], in_=c_cvi[:])
            c_wv = cp.tile([128, NT], F32)
            if wv_wire:
                c_wv16 = cp.tile([128, NT], BF16)
                nc.sync.dma_start(c_wv16[:], blob[:, 2 * NT:3 * NT])
                nc.vector.tensor_copy(out=c_wv[:], in_=c_wv16[:])
            else:
                c_cnti = cp.tile([128, NT], I32)
                nc.vector.tensor_scalar(out=c_cnti[:], in0=c_praw[:],
                                        scalar1=26, scalar2=None,
                                        op0=mybir.AluOpType.logical_shift_right)
                c_cntf = cp.tile([128, NT], F32)
                nc.vector.tensor_copy(out=c_cntf[:], in_=c_cnti[:])
                nc.vector.reciprocal(out=c_wv[:], in_=c_cntf[:])
            c_ioti = cp.tile([128, 256], I32)
            nc.gpsimd.iota(c_ioti[:], pattern=[[1, 256]], base=0,
                           channel_multiplier=0)
            c_iota = cp.tile([128, 256], F32)
            nc.vector.tensor_copy(out=c_iota[:], in_=c_ioti[:])
            c_W = cp.tile([128, W_W], BF16)
            nc.sync.dma_start(c_W[:], blob[:, oW:oW + W_W])
            c_Wroot = c_W[:, 0:128]
            c_Wrel0 = c_W[:, 128:256]
            c_Wrel1 = c_W[:, 256:384]
            c_Wo1 = c_W[:, 384:512]
            c_Wo2 = c_W[:, 512:514]
            c_f32 = cp.tile([128, 4], F32)
            nc.sync.dma_start(c_f32[:], blob[:, oF:oF + 8].bitcast(F32))
            c_brg = c_f32[:, 0:1]
            c_bo1 = c_f32[:, 1:2]
            c_xs = c_f32[:, 2:3]
            c_bo2 = c_f32[0:2, 3:4]
            ident = cp.tile([128, 128], BF16)
            make_identity(nc, ident[:])

            # DRAM intermediates
            xfm = [dp.tile([128, SHARD], BF16, name=f"xfm{i}") for i in range(3)]
            xnm = [dp.tile([SHARD, 128], BF16, name=f"xnm{i}") for i in range(2)]
            tables = [dp.tile([TROWS, 128], BF16, addr_space="Shared", name=f"table{i}")
                      for i in range(2)]

            # ---------------- ingest: int8 x -> bf16 table + scaled fm ----------
            with tc.tile_pool(name="ing", bufs=4) as ip, \
                 tc.tile_pool(name="ingps", bufs=2, space="PSUM") as ips:
                for k in range(BPC):
                    nm8 = ip.tile([128, 128], I8, name="nm8")
                    nc.sync.dma_start(nm8[:], xq[k * 128:(k + 1) * 128, :])
                    nmb = ip.tile([128, 128], BF16, name="nmb")
                    nc.vector.tensor_copy(out=nmb[:], in_=nm8[:])
                    nc.sync.dma_start(xnm[0][k * 128:(k + 1) * 128, :], nmb[:])
                    ps_t = ips.tile([128, 128], BF16, name="ps_t")
                    nc.tensor.matmul(out=ps_t[:], lhsT=nmb[:], rhs=ident[:],
                                     is_transpose=True, start=True, stop=True)
                    fm = ip.tile([128, 128], BF16, name="fm")
                    nc.vector.tensor_scalar(out=fm[:], in0=ps_t[:],
                                            scalar1=c_xs, scalar2=None,
                                            op0=mybir.AluOpType.mult)
                    nc.sync.dma_start(xfm[0][:, k * 128:(k + 1) * 128], fm[:])

            nc.gpsimd.collective_compute(AG, BY, replica_groups=groups,
                                         ins=[xnm[0].opt()], outs=[tables[0].opt()])

            # ---------------- rgcn layers ----------------
            for L in range(2):
                table, xin, xout = tables[L], xfm[L], xfm[L + 1]
                with tc.tile_pool(name=f"gp{L}", bufs=16) as gp, \
                     tc.tile_pool(name=f"sp{L}", bufs=8) as sp, \
                     tc.tile_pool(name=f"up{L}", bufs=2) as up, \
                     tc.tile_pool(name=f"Sps{L}", bufs=4, space="PSUM") as Sps, \
                     tc.tile_pool(name=f"aps{L}", bufs=2, space="PSUM") as aps, \
                     tc.tile_pool(name=f"tps{L}", bufs=2, space="PSUM") as tps:
                    n_units = BPC // 2
                    for u in range(n_units):
                        psS = []
                        for h in range(2):
                            b = u * 2 + h
                            ps = Sps.tile([128, 256], F32, name="psS")
                            psS.append(ps)
                            for t in range(T_pad):
                                T = b * T_pad + t
                                G = gp.tile([128, 128], BF16, name="G")
                                nc.gpsimd.indirect_dma_start(
                                    out=G[:], out_offset=None, in_=table[:],
                                    in_offset=bass.IndirectOffsetOnAxis(
                                        ap=c_gidx[:, T:T + 1], axis=0))
                                sel = sp.tile([128, 256], BF16, name="sel")
                                nc.vector.tensor_scalar(
                                    out=sel[:], in0=c_iota[:],
                                    scalar1=c_cv[:, T:T + 1], scalar2=c_wv[:, T:T + 1],
                                    op0=mybir.AluOpType.is_equal,
                                    op1=mybir.AluOpType.mult)
                                nc.tensor.matmul(out=ps[:], lhsT=G[:], rhs=sel[:],
                                                 start=(t == 0), stop=(t == T_pad - 1))
                        # unit tail: transforms for 2 blocks (256 dst cols)
                        U0 = up.tile([128, 256], BF16, name="U0")
                        U1 = up.tile([128, 256], BF16, name="U1")
                        for h in range(2):
                            if L == 0:
                                # apply int8 dequant scale per feature
                                nc.vector.tensor_scalar(
                                    out=U0[:, h * 128:(h + 1) * 128],
                                    in0=psS[h][:, 0:128], scalar1=c_xs,
                                    scalar2=None, op0=mybir.AluOpType.mult)
                                nc.vector.tensor_scalar(
                                    out=U1[:, h * 128:(h + 1) * 128],
                                    in0=psS[h][:, 128:256], scalar1=c_xs,
                                    scalar2=None, op0=mybir.AluOpType.mult)
                            else:
                                nc.vector.tensor_copy(
                                    out=U0[:, h * 128:(h + 1) * 128],
                                    in_=psS[h][:, 0:128])
                                nc.vector.tensor_copy(
                                    out=U1[:, h * 128:(h + 1) * 128],
                                    in_=psS[h][:, 128:256])
                        xr = up.tile([128, 256], BF16, name="xr")
                        nc.sync.dma_start(xr[:], xin[:, u * 256:(u + 1) * 256])
                        agg = aps.tile([128, 256], F32, name="agg")
                        nc.tensor.matmul(out=agg[:], lhsT=c_Wroot, rhs=xr[:],
                                         start=True, stop=False)
                        nc.tensor.matmul(out=agg[:], lhsT=c_Wrel0, rhs=U0[:],
                                         start=False, stop=False)
                        nc.tensor.matmul(out=agg[:], lhsT=c_Wrel1, rhs=U1[:],
                                         start=False, stop=True)
                        y = up.tile([128, 256], BF16, name="y")
                        nc.scalar.activation(out=y[:], in_=agg[:],
                                             func=mybir.ActivationFunctionType.Identity,
                                             bias=c_brg, scale=1.0)
                        nc.sync.dma_start(xout[:, u * 256:(u + 1) * 256], y[:])
                        if L == 0:
                            for j in range(2):
                                ps_t = tps.tile([128, 128], BF16, name="ps_t2")
                                nc.tensor.matmul(
                                    out=ps_t[:],
                                    lhsT=y[:, j * 128:(j + 1) * 128],
                                    rhs=ident[:], is_transpose=True,
                                    start=True, stop=True)
                                tr_t = up.tile([128, 128], BF16, name="tr2")
                                nc.vector.tensor_copy(out=tr_t[:], in_=ps_t[:])
                                nc.sync.dma_start(
                                    xnm[1][u * 256 + j * 128:u * 256 + (j + 1) * 128, :],
                                    tr_t[:])
                if L == 0:
                    nc.gpsimd.collective_compute(AG, BY, replica_groups=groups,
                                                 ins=[xnm[1].opt()],
                                                 outs=[tables[1].opt()])

            # ---------------- head ----------------
            with tc.tile_pool(name="hd", bufs=3) as hp, \
                 tc.tile_pool(name="hps", bufs=2, space="PSUM") as hps:
                for (c0, w) in _enc_slices(SHARD):
                    xt = hp.tile([128, w], BF16, name="xt")
                    nc.sync.dma_start(xt[:], xfm[2][:, c0:c0 + w])
                    ps_h = hps.tile([128, w], F32, name="ps_h")
                    nc.tensor.matmul(out=ps_h[:], lhsT=c_Wo1, rhs=xt[:],
                                     start=True, stop=True)
                    z_t = _lrelu(hp, ps_h[:], c_bo1, w, "z_t")
                    ps_o = hps.tile([2, w], F32, name="ps_o")
                    nc.tensor.matmul(out=ps_o[:], lhsT=c_Wo2, rhs=z_t[:],
                                     start=True, stop=True)
                    o_t = hp.tile([2, w], BF16, name="o_t")
                    nc.scalar.activation(out=o_t[:], in_=ps_o[:],
                                         func=mybir.ActivationFunctionType.Identity,
                                         bias=c_bo2, scale=1.0)
                    nc.sync.dma_start(out[:, c0:c0 + w], o_t[:])
    nc.compile()
    return nc


# ----------------------------------------------------------------------------
# cached PJRT runner (jit trace + NEFF compile + device load happen once)
# ----------------------------------------------------------------------------

class _Runner:
    def __init__(self, cfg):
        self.cfg = cfg
        self.nc = build_bass(cfg)
        b2j.install_neuronx_cc_hook()
        nc = self.nc
        partition_name = (nc.partition_id_tensor.name
                          if nc.partition_id_tensor else None)
        in_names, out_names, out_avals = [], [], []
        for alloc in nc.m.functions[0].allocations:
            if not isinstance(alloc, mybir.MemoryLocationSet):
                continue
            name = alloc.memorylocations[0].name
            if alloc.kind == "ExternalInput":
                if name != partition_name:
                    in_names.append(name)
            elif alloc.kind == "ExternalOutput":
                shape = tuple(alloc.tensor_shape)
                dtype = mybir.dt.np(alloc.dtype)
                out_names.append(name)
                out_avals.append(jax.core.ShapedArray(shape, dtype))
        self.in_names = list(in_names)
        self.out_names = out_names
        self.out_avals = out_avals
        n_params = len(in_names)
        n_outs = len(out_avals)
        bind_names = in_names + out_names
        if partition_name is not None:
            bind_names = bind_names + [partition_name]

        def _body(*args):
            operands = list(args)
            if partition_name is not None:
                operands.append(b2j.partition_id_tensor())
            outs = b2j._bass_exec_p.bind(
                *operands,
                out_avals=tuple(out_avals),
                in_names=tuple(bind_names),
                out_names=tuple(out_names),
                lowering_input_output_aliases=(),
                sim_require_finite=True,
                sim_require_nnan=True,
                nc=nc,
            )
            return tuple(outs)

        devices = jax.devices()[:N_CORES]
        mesh = Mesh(np.asarray(devices), ("core",))
        in_specs = (PartitionSpec("core"),) * (n_params + n_outs)
        out_specs = (PartitionSpec("core"),) * n_outs
        self.sharded = jax.jit(
            shard_map(_body, mesh=mesh, in_specs=in_specs, out_specs=out_specs,
                      check_rep=False),
            donate_argnums=tuple(range(n_params, n_params + n_outs)),
            keep_unused=True,
        )

    def run_global(self, global_in):
        """global_in: name -> [N_CORES*rows, ...] array (no per-core concat)."""
        concat_in = [np.ascontiguousarray(global_in[n]) for n in self.in_names]
        concat_zeros = [
            np.zeros((N_CORES * a.shape[0], *a.shape[1:]), a.dtype)
            for a in self.out_avals
        ]
        outs = self.sharded(*concat_in, *concat_zeros)
        return [
            {name: np.asarray(outs[i]).reshape(N_CORES, *self.out_avals[i].shape)[c]
             for i, name in enumerate(self.out_names)}
            for c in range(N_CORES)
        ]

    def __call__(self, maps):
        return self.run_global({
            n: np.concatenate([np.asarray(m[n]) for m in maps], axis=0)
            for n in self.in_names
        })


_RUNNERS = {}


def _get_runner(cfg):
    key = (cfg["N"], cfg["E"], cfg["T_pad"], cfg["wv_wire"])
    r = _RUNNERS.get(key)
    if r is None:
        r = _Runner(cfg)
        _RUNNERS[key] = r
    return r


# ----------------------------------------------------------------------------
# entry point
# ----------------------------------------------------------------------------

def _in_maps(cfg, per_core):
    return [dict(blob=per_core["big"][c], xq=per_core["xq"][c])
            for c in range(N_CORES)]


def _global_in(cfg, per_core):
    # contiguous [8, r, c] -> [8*r, c] reshapes: zero-copy views
    big = per_core["big"]
    xq = per_core["xq"]
    return dict(blob=big.reshape(-1, big.shape[-1]),
                xq=xq.reshape(-1, xq.shape[-1]))


def _assemble(cfg, asm, core_outs):
    stacked = np.stack([co["out"] for co in core_outs])      # [8, 2, SHARD]
    out = stacked[asm["node_core"], :, asm["node_pos"]]       # [N, 2]
    return np.ascontiguousarray(out.astype(np.float32))


def kernel(**inputs):
    cfg, per_core, asm = _prep(inputs)
    runner = _get_runner(cfg)
    res = runner.run_global(_global_in(cfg, per_core))
    return _assemble(cfg, asm, res)


# revision 22
# speedup vs baseline: 12.4907x; 1.0072x over previous
"""BotRGCN on 8 Trainium2 NeuronCores (Bass/Tile).

Strategy (1-D destination-sharded graph partition):
  - Host assigns nodes to 8*BPC blocks of 128 destinations via sorted-serpentine
    balancing on in-degree, so every block holds ~E/(8*BPC) edges; shard s =
    blocks [s*BPC, (s+1)*BPC).  Edges are bucketed per (core, dst-block) and
    padded to a uniform T_pad tiles of 128 edges.
  - The tiny encoder MLP (4.7 GFLOP) runs on host in f32; nodes ship as the
    128-d embedding x quantized int8 with per-feature scales (the wall-clock
    is dominated by the tunneled host->device link, so wire bytes rule).
  - Each core: int8 x -> bf16 (raw ints) -> AllGather into a replicated bf16
    gather table; PE transposes build the feature-major copy with the
    dequant scale applied per partition.  Per dst-block: per-tile
    indirect-DMA row gather + DVE weighted one-hot (tensor_scalar
    is_equal*mult) + scatter-matmul accumulating relation-split sums in
    PSUM (f32); per 2-block unit: W_rel / W_root transform matmuls + bias.
    Layer-1 U copies apply the int8 dequant scale; layer 2 runs true-scale.
    Head = two matmuls + Prelu; per-core output [2, SHARD], host
    inverse-permutes to [N, 2].
  - Edge metadata ships as ONE int32 per edge slot: gather row (17 bits) |
    dst (lane,rel) selector cv (9 bits, 511 = padding sentinel) | mean
    count (6 bits, wv = 1/cnt via DVE reciprocal on device).  Everything
    except the int8 x ships as a single bf16 blob per core (edge words
    bitcast, weights, biases, scales) to minimize per-transfer overhead.
    The compiled executable + jit trace are cached across calls, so each
    call pays only transfer + execute.
"""

import numpy as np
import ml_dtypes

import jax
from jax.sharding import Mesh, PartitionSpec
from jax.experimental.shard_map import shard_map

import concourse.bacc as bacc
import concourse.bass as bass
import concourse.bass2jax as b2j
import concourse.mybir as mybir
import concourse.tile as tile
from concourse.masks import make_identity

F32 = mybir.dt.float32
BF16 = mybir.dt.bfloat16
I32 = mybir.dt.int32
I8 = mybir.dt.int8
BF = ml_dtypes.bfloat16

N_CORES = 8
D = 128
R = 2
ALPHA = 0.01
W_W = 514           # weight section cols: Wroot|Wrel0|Wrel1|Wo1 (4*128) + Wo2 (2)


def _lrelu_np(v):
    return np.where(v > 0, v, np.float32(ALPHA) * v)


def _blob_width(NT, wv_wire):
    return 2 * NT + (NT if wv_wire else 0) + W_W + 8


# ----------------------------------------------------------------------------
# host-side preprocessing (vectorized): graph partition + encoder + quant
# ----------------------------------------------------------------------------

def _prep(inputs):
    src = np.asarray(inputs["edge_index"][0], dtype=np.int64)
    dst = np.asarray(inputs["edge_index"][1], dtype=np.int64)
    rel = np.asarray(inputs["edge_type"], dtype=np.int64)
    N = int(np.asarray(inputs["des"]).shape[0])
    E = src.shape[0]

    BPC = (-(-N // N_CORES) + 127) // 128  # ceil(ceil(N/8)/128)
    SHARD = BPC * 128
    NBLK = N_CORES * BPC
    TROWS = N_CORES * SHARD
    assert N <= NBLK * 128
    assert TROWS <= (1 << 17), "gather row must fit 17 bits"

    # per-(dst,rel) counts -> mean weights;  per-dst totals for balancing
    cnt = np.bincount(dst * R + rel, minlength=N * R)
    deg = cnt.reshape(N, R).sum(1)

    # sorted-serpentine: nodes by degree desc, dealt across NBLK blocks
    # alternating direction each round -> near-optimal edge balance.
    order = np.argsort(-deg, kind="stable")
    idx = np.arange(N)
    rnd = idx // NBLK
    pos = idx % NBLK
    blk = np.where(rnd % 2 == 0, pos, NBLK - 1 - pos)
    node_block = np.empty(N, np.int64)
    node_lane = np.empty(N, np.int64)
    node_block[order] = blk
    node_lane[order] = rnd

    node_core = node_block // BPC
    node_pos = (node_block % BPC) * 128 + node_lane      # position in shard
    node_row = node_core * SHARD + node_pos              # row in gather table

    # edge buckets keyed by destination block
    key = node_block[dst]
    bucket_cnt = np.bincount(key, minlength=NBLK)
    T_pad = int(-(-bucket_cnt.max() // 128))

    CAP = T_pad * 128
    order_e = np.argsort(key, kind="stable")
    ks = key[order_e]
    start = np.zeros(NBLK, np.int64)
    start[1:] = np.cumsum(bucket_cnt)[:-1]
    pos_in_bucket = np.arange(E) - start[ks]
    slot = ks * CAP + pos_in_bucket                      # global slot id

    # one int32 per edge slot: row | cv<<17 | cnt<<26
    # padding: cv=511 (never matches the 0..255 iota), cnt=1 (finite 1/cnt)
    se, de, re_ = src[order_e], dst[order_e], rel[order_e]
    cv = re_ * 128 + node_lane[de]
    cntv = cnt[de * R + re_]
    wv_wire = bool(cntv.max() > 63)
    packed = np.full(NBLK * CAP, np.int32((511 << 17) | (1 << 26)), np.int32)
    if wv_wire:
        packed[slot] = (node_row[se] | (cv << 17) | (1 << 26)).astype(np.int32)
    else:
        packed[slot] = (node_row[se] | (cv << 17) | (cntv << 26)).astype(np.int32)

    # reshape to per-core SBUF layouts [128, BPC*T_pad]
    def to_sbuf(a):
        # [NBLK, T_pad, 128] -> per core [128, BPC*T_pad]
        a = a.reshape(N_CORES, BPC, T_pad, 128)
        return np.ascontiguousarray(a.transpose(0, 3, 1, 2).reshape(N_CORES, 128, BPC * T_pad))

    packed_c = to_sbuf(packed)
    NT = BPC * T_pad

    # host encoder: x = lrelu(W_in . lrelu(enc(feat)) + b_in)   [N, 128] f32
    g = lambda k: np.asarray(inputs[k], np.float32)
    d = _lrelu_np(g("des") @ g("W_des") + g("b_des"))
    t = _lrelu_np(g("tweet") @ g("W_tweet") + g("b_tweet"))
    n = _lrelu_np(g("num_prop") @ g("W_num") + g("b_num"))
    c = _lrelu_np(g("cat_prop") @ g("W_cat") + g("b_cat"))
    x0 = np.concatenate([d, t, n, c], axis=1)
    x = _lrelu_np(x0 @ g("W_in") + g("b_in"))

    # per-feature int8 quantization
    xs = np.maximum(np.abs(x).max(axis=0), 1e-12).astype(np.float32) / 127.0
    xq = np.clip(np.rint(x / xs), -127, 127).astype(np.int8)

    # permute into table order, node-major per core [SHARD, 128]
    row_node = np.full(TROWS, -1, np.int64)
    row_node[node_row] = np.arange(N)
    valid = row_node >= 0
    safe = np.where(valid, row_node, 0)
    Xr = xq[safe]
    Xr[~valid] = 0
    xq_c = np.ascontiguousarray(Xr.reshape(N_CORES, SHARD, 128))

    # the bf16 blob: packed (bitcast) | [wv] | weights | f32 biases+scales
    W_BIG = _blob_width(NT, wv_wire)
    oW = W_BIG - W_W - 8
    oF = W_BIG - 8
    big = np.zeros((N_CORES, 128, W_BIG), BF)
    big[:, :, 0:2 * NT] = packed_c.view(BF)
    if wv_wire:
        wv = np.zeros(NBLK * CAP, np.float32)
        wv[slot] = (1.0 / cntv).astype(np.float32)
        big[:, :, 2 * NT:3 * NT] = to_sbuf(wv).astype(BF)
    wrel = g("W_rel")
    Wsec = np.concatenate(
        [g("W_root"), wrel[0], wrel[1], g("W_o1"), g("W_o2")], axis=1).astype(BF)
    big[:, :, oW:oW + W_W] = Wsec
    f32sec = np.zeros((128, 4), np.float32)
    f32sec[:, 0] = g("b_rgcn")
    f32sec[:, 1] = g("b_o1")
    f32sec[:, 2] = xs
    f32sec[0:2, 3] = g("b_o2")
    big[:, :, oF:oF + 8] = f32sec.view(BF)

    cfg = dict(N=N, E=E, BPC=BPC, SHARD=SHARD, TROWS=TROWS, T_pad=T_pad,
               wv_wire=wv_wire)
    per_core = dict(big=big, xq=xq_c)
    asm = dict(node_core=node_core, node_pos=node_pos)
    return cfg, per_core, asm


# ----------------------------------------------------------------------------
# device program
# ----------------------------------------------------------------------------

def _enc_slices(shard):
    out, c = [], 0
    while c < shard:
        w = min(512, shard - c)
        out.append((c, w))
        c += w
    return out


def build_bass(cfg, sim_compat=False):
    BPC, SHARD, TROWS, T_pad = cfg["BPC"], cfg["SHARD"], cfg["TROWS"], cfg["T_pad"]
    wv_wire = cfg["wv_wire"]
    NT = BPC * T_pad
    W_BIG = _blob_width(NT, wv_wire)
    oW = W_BIG - W_W - 8
    oF = W_BIG - 8
    nc = bacc.Bacc("TRN2", target_bir_lowering=False, debug=False,
                   num_devices=N_CORES)

    blob = nc.dram_tensor("blob", [128, W_BIG], BF16, kind="ExternalInput")
    xq = nc.dram_tensor("xq", [SHARD, 128], I8, kind="ExternalInput")
    out = nc.dram_tensor("out", [2, SHARD], BF16, kind="ExternalOutput")

    groups = [list(range(N_CORES))]
    AG = "AllGather"
    BY = mybir.AluOpType.bypass

    def _lrelu(pool, ps_ap, bias_ap, w, name):
        t = pool.tile([ps_ap.shape[0], w], BF16, name=name)
        if not sim_compat:
            nc.scalar.activation(out=t[:], in_=ps_ap,
                                 func=mybir.ActivationFunctionType.Prelu,
                                 bias=bias_ap, scale=1.0, alpha=ALPHA)
            return t
        zt = pool.tile([ps_ap.shape[0], w], F32, name=name + "_z")
        nc.scalar.activation(out=zt[:], in_=ps_ap,
                             func=mybir.ActivationFunctionType.Identity,
                             bias=bias_ap, scale=1.0)
        rt = pool.tile([ps_ap.shape[0], w], F32, name=name + "_r")
        nc.scalar.activation(out=rt[:], in_=ps_ap,
                             func=mybir.ActivationFunctionType.Relu,
                             bias=bias_ap, scale=1.0)
        t1 = pool.tile([ps_ap.shape[0], w], F32, name=name + "_t1")
        nc.vector.tensor_scalar(out=t1[:], in0=zt[:], scalar1=ALPHA, scalar2=None,
                                op0=mybir.AluOpType.mult)
        t2 = pool.tile([ps_ap.shape[0], w], F32, name=name + "_t2")
        nc.vector.tensor_scalar(out=t2[:], in0=rt[:], scalar1=1.0 - ALPHA, scalar2=None,
                                op0=mybir.AluOpType.mult)
        nc.vector.tensor_tensor(out=t[:], in0=t1[:], in1=t2[:],
                                op=mybir.AluOpType.add)
        return t

    with tile.TileContext(nc) as tc:
        with tc.tile_pool(name="const", bufs=1) as cp, \
             tc.tile_pool(name="dram", bufs=1, space="DRAM") as dp:
            # unpack the blob
            c_praw = cp.tile([128, NT], I32)
            nc.sync.dma_start(c_praw[:], blob[:, 0:2 * NT].bitcast(I32))
            c_gidx = cp.tile([128, NT], I32)
            nc.vector.tensor_scalar(out=c_gidx[:], in0=c_praw[:],
                                    scalar1=0x1FFFF, scalar2=None,
                                    op0=mybir.AluOpType.bitwise_and)
            c_cvi = cp.tile([128, NT], I32)
            nc.vector.tensor_scalar(out=c_cvi[:], in0=c_praw[:],
                                    scalar1=17, scalar2=0x1FF,
                                    op0=mybir.AluOpType.logical_shift_right,
                                    op1=mybir.AluOpType.bitwise_and)
            c_cv = cp.tile([128, NT], F32)
            nc.vector.tensor_copy(out=c_cv[:], in_=c_cvi[:])
            c_wv = cp.tile([128, NT], F32)
            if wv_wire:
                c_wv16 = cp.tile([128, NT], BF16)
                nc.sync.dma_start(c_wv16[:], blob[:, 2 * NT:3 * NT])
                nc.vector.tensor_copy(out=c_wv[:], in_=c_wv16[:])
            else:
                c_cnti = cp.tile([128, NT], I32)
                nc.vector.tensor_scalar(out=c_cnti[:], in0=c_praw[:],
                                        scalar1=26, scalar2=None,
                                        op0=mybir.AluOpType.logical_shift_right)
                c_cntf = cp.tile([128, NT], F32)
                nc.vector.tensor_copy(out=c_cntf[:], in_=c_cnti[:])
                nc.vector.reciprocal(out=c_wv[:], in_=c_cntf[:])
            c_ioti = cp.tile([128, 256], I32)
            nc.gpsimd.iota(c_ioti[:], pattern=[[1, 256]], base=0,
                           channel_multiplier=0)
            c_iota = cp.tile([128, 256], F32)
            nc.vector.tensor_copy(out=c_iota[:], in_=c_ioti[:])
            c_W = cp.tile([128, W_W], BF16)
            nc.sync.dma_start(c_W[:], blob[:, oW:oW + W_W])
            c_Wroot = c_W[:, 0:128]
            c_Wrel0 = c_W[:, 128:256]
            c_Wrel1 = c_W[:, 256:384]
            c_Wo1 = c_W[:, 384:512]
            c_Wo2 = c_W[:, 512:514]
            c_f32 = cp.tile([128, 4], F32)
            nc.sync.dma_start(c_f32[:], blob[:, oF:oF + 8].bitcast(F32))
            c_brg = c_f32[:, 0:1]
            c_bo1 = c_f32[:, 1:2]
            c_xs = c_f32[:, 2:3]
            c_bo2 = c_f32[0:2, 3:4]
            ident = cp.tile([128, 128], BF16)
            make_identity(nc, ident[:])

            # DRAM intermediates
            xfm = [dp.tile([128, SHARD], BF16, name=f"xfm{i}") for i in range(3)]
            xnm = [dp.tile([SHARD, 128], BF16, name=f"xnm{i}") for i in range(2)]
            tables = [dp.tile([TROWS, 128], BF16, addr_space="Shared", name=f"table{i}")
                      for i in range(2)]

            # ---------------- ingest: int8 x -> bf16 table + scaled fm ----------
            with tc.tile_pool(name="ing", bufs=4) as ip, \
                 tc.tile_pool(name="ingps", bufs=2, space="PSUM") as ips:
                for k in range(BPC):
                    nm8 = ip.tile([128, 128], I8, name="nm8")
                    nc.sync.dma_start(nm8[:], xq[k * 128:(k + 1) * 128, :])
                    nmb = ip.tile([128, 128], BF16, name="nmb")
                    nc.vector.tensor_copy(out=nmb[:], in_=nm8[:])
                    nc.sync.dma_start(xnm[0][k * 128:(k + 1) * 128, :], nmb[:])
                    ps_t = ips.tile([128, 128], BF16, name="ps_t")
                    nc.tensor.matmul(out=ps_t[:], lhsT=nmb[:], rhs=ident[:],
                                     is_transpose=True, start=True, stop=True)
                    fm = ip.tile([128, 128], BF16, name="fm")
                    nc.vector.tensor_scalar(out=fm[:], in0=ps_t[:],
                                            scalar1=c_xs, scalar2=None,
                                            op0=mybir.AluOpType.mult)
                    nc.sync.dma_start(xfm[0][:, k * 128:(k + 1) * 128], fm[:])

            nc.gpsimd.collective_compute(AG, BY, replica_groups=groups,
                                         ins=[xnm[0].opt()], outs=[tables[0].opt()])

            # ---------------- rgcn layers ----------------
            for L in range(2):
                table, xin, xout = tables[L], xfm[L], xfm[L + 1]
                with tc.tile_pool(name=f"gp{L}", bufs=16) as gp, \
                     tc.tile_pool(name=f"sp{L}", bufs=8) as sp, \
                     tc.tile_pool(name=f"up{L}", bufs=2) as up, \
                     tc.tile_pool(name=f"Sps{L}", bufs=4, space="PSUM") as Sps, \
                     tc.tile_pool(name=f"aps{L}", bufs=2, space="PSUM") as aps, \
                     tc.tile_pool(name=f"tps{L}", bufs=2, space="PSUM") as tps:
                    n_units = BPC // 2
                    for u in range(n_units):
                        psS = []
                        for h in range(2):
                            b = u * 2 + h
                            ps = Sps.tile([128, 256], F32, name="psS")
                            psS.append(ps)
                            for t in range(T_pad):
                                T = b * T_pad + t
                                G = gp.tile([128, 128], BF16, name="G")
                                nc.gpsimd.indirect_dma_start(
                                    out=G[:], out_offset=None, in_=table[:],
                                    in_offset=bass.IndirectOffsetOnAxis(
                                        ap=c_gidx[:, T:T + 1], axis=0))
                                sel = sp.tile([128, 256], BF16, name="sel")
                                nc.vector.tensor_scalar(
                                    out=sel[:], in0=c_iota[:],
                                    scalar1=c_cv[:, T:T + 1], scalar2=c_wv[:, T:T + 1],
                                    op0=mybir.AluOpType.is_equal,
                                    op1=mybir.AluOpType.mult)
                                nc.tensor.matmul(out=ps[:], lhsT=G[:], rhs=sel[:],
                                                 start=(t == 0), stop=(t == T_pad - 1))
                        # unit tail: transforms for 2 blocks (256 dst cols)
                        U0 = up.tile([128, 256], BF16, name="U0")
                        U1 = up.tile([128, 256], BF16, name="U1")
                        for h in range(2):
                            if L == 0:
                                # apply int8 dequant scale per feature
                                nc.vector.tensor_scalar(
                                    out=U0[:, h * 128:(h + 1) * 128],
                                    in0=psS[h][:, 0:128], scalar1=c_xs,
                                    scalar2=None, op0=mybir.AluOpType.mult)
                                nc.vector.tensor_scalar(
                                    out=U1[:, h * 128:(h + 1) * 128],
                                    in0=psS[h][:, 128:256], scalar1=c_xs,
                                    scalar2=None, op0=mybir.AluOpType.mult)
                            else:
                                nc.vector.tensor_copy(
                                    out=U0[:, h * 128:(h + 1) * 128],
                                    in_=psS[h][:, 0:128])
                                nc.vector.tensor_copy(
                                    out=U1[:, h * 128:(h + 1) * 128],
                                    in_=psS[h][:, 128:256])
                        xr = up.tile([128, 256], BF16, name="xr")
                        nc.sync.dma_start(xr[:], xin[:, u * 256:(u + 1) * 256])
                        agg = aps.tile([128, 256], F32, name="agg")
                        nc.tensor.matmul(out=agg[:], lhsT=c_Wroot, rhs=xr[:],
                                         start=True, stop=False)
                        nc.tensor.matmul(out=agg[:], lhsT=c_Wrel0, rhs=U0[:],
                                         start=False, stop=False)
                        nc.tensor.matmul(out=agg[:], lhsT=c_Wrel1, rhs=U1[:],
                                         start=False, stop=True)
                        y = up.tile([128, 256], BF16, name="y")
                        nc.scalar.activation(out=y[:], in_=agg[:],
                                             func=mybir.ActivationFunctionType.Identity,
                                             bias=c_brg, scale=1.0)
                        nc.sync.dma_start(xout[:, u * 256:(u + 1) * 256], y[:])
                        if L == 0:
                            for j in range(2):
                                ps_t = tps.tile([128, 128], BF16, name="ps_t2")
                                nc.tensor.matmul(
                                    out=ps_t[:],
                                    lhsT=y[:, j * 128:(j + 1) * 128],
                                    rhs=ident[:], is_transpose=True,
                                    start=True, stop=True)
                                tr_t = up.tile([128, 128], BF16, name="tr2")
                                nc.vector.tensor_copy(out=tr_t[:], in_=ps_t[:])
                                nc.sync.dma_start(
                                    xnm[1][u * 256 + j * 128:u * 256 + (j + 1) * 128, :],
                                    tr_t[:])
                if L == 0:
                    nc.gpsimd.collective_compute(AG, BY, replica_groups=groups,
                                                 ins=[xnm[1].opt()],
                                                 outs=[tables[1].opt()])

            # ---------------- head ----------------
            with tc.tile_pool(name="hd", bufs=3) as hp, \
                 tc.tile_pool(name="hps", bufs=2, space="PSUM") as hps:
                for (c0, w) in _enc_slices(SHARD):
                    xt = hp.tile([128, w], BF16, name="xt")
                    nc.sync.dma_start(xt[:], xfm[2][:, c0:c0 + w])
                    ps_h = hps.tile([128, w], F32, name="ps_h")
                    nc.tensor.matmul(out=ps_h[:], lhsT=c_Wo1, rhs=xt[:],
                                     start=True, stop=True)
                    z_t = _lrelu(hp, ps_h[:], c_bo1, w, "z_t")
                    ps_o = hps.tile([2, w], F32, name="ps_o")
                    nc.tensor.matmul(out=ps_o[:], lhsT=c_Wo2, rhs=z_t[:],
                                     start=True, stop=True)
                    o_t = hp.tile([2, w], BF16, name="o_t")
                    nc.scalar.activation(out=o_t[:], in_=ps_o[:],
                                         func=mybir.ActivationFunctionType.Identity,
                                         bias=c_bo2, scale=1.0)
                    nc.sync.dma_start(out[:, c0:c0 + w], o_t[:])
    nc.compile()
    return nc


# ----------------------------------------------------------------------------
# cached PJRT runner (jit trace + NEFF compile + device load happen once)
# ----------------------------------------------------------------------------

class _Runner:
    def __init__(self, cfg):
        self.cfg = cfg
        self.nc = build_bass(cfg)
        b2j.install_neuronx_cc_hook()
        nc = self.nc
        partition_name = (nc.partition_id_tensor.name
                          if nc.partition_id_tensor else None)
        in_names, out_names, out_avals = [], [], []
        for alloc in nc.m.functions[0].allocations:
            if not isinstance(alloc, mybir.MemoryLocationSet):
                continue
            name = alloc.memorylocations[0].name
            if alloc.kind == "ExternalInput":
                if name != partition_name:
                    in_names.append(name)
            elif alloc.kind == "ExternalOutput":
                shape = tuple(alloc.tensor_shape)
                dtype = mybir.dt.np(alloc.dtype)
                out_names.append(name)
                out_avals.append(jax.core.ShapedArray(shape, dtype))
        self.in_names = list(in_names)
        self.out_names = out_names
        self.out_avals = out_avals
        n_params = len(in_names)
        n_outs = len(out_avals)
        bind_names = in_names + out_names
        if partition_name is not None:
            bind_names = bind_names + [partition_name]

        def _body(*args):
            operands = list(args)
            if partition_name is not None:
                operands.append(b2j.partition_id_tensor())
            outs = b2j._bass_exec_p.bind(
                *operands,
                out_avals=tuple(out_avals),
                in_names=tuple(bind_names),
                out_names=tuple(out_names),
                lowering_input_output_aliases=(),
                sim_require_finite=True,
                sim_require_nnan=True,
                nc=nc,
            )
            return tuple(outs)

        devices = jax.devices()[:N_CORES]
        mesh = Mesh(np.asarray(devices), ("core",))
        in_specs = (PartitionSpec("core"),) * (n_params + n_outs)
        out_specs = (PartitionSpec("core"),) * n_outs
        # The "out" operands are never read (the kernel writes every element of
        # every output): pass permanent device-resident dummies, NOT donated,
        # so they are not re-uploaded on every call.
        self.sharded = jax.jit(
            shard_map(_body, mesh=mesh, in_specs=in_specs, out_specs=out_specs,
                      check_rep=False),
            keep_unused=True,
        )
        shard_sp = jax.sharding.NamedSharding(mesh, PartitionSpec("core"))
        self.dev_dummy = [
            jax.device_put(
                np.zeros((N_CORES * a.shape[0], *a.shape[1:]), a.dtype), shard_sp)
            for a in self.out_avals
        ]
        from concurrent.futures import ThreadPoolExecutor
        self._pool = ThreadPoolExecutor(max_workers=N_CORES)

    def _fetch(self, arr):
        # per-shard D2H round trips overlap across threads
        shards = arr.addressable_shards
        parts = list(self._pool.map(
            lambda s: ((s.index[0].start or 0), np.asarray(s.data)), shards))
        parts.sort(key=lambda t: t[0])
        return np.concatenate([p[1] for p in parts], axis=0)

    def run_global(self, global_in):
        """global_in: name -> [N_CORES*rows, ...] array (no per-core concat)."""
        concat_in = [np.ascontiguousarray(global_in[n]) for n in self.in_names]
        outs = self.sharded(*concat_in, *self.dev_dummy)
        fetched = [self._fetch(outs[i]).reshape(N_CORES, *self.out_avals[i].shape)
                   for i in range(len(self.out_names))]
        return [
            {name: fetched[i][c] for i, name in enumerate(self.out_names)}
            for c in range(N_CORES)
        ]

    def __call__(self, maps):
        return self.run_global({
            n: np.concatenate([np.asarray(m[n]) for m in maps], axis=0)
            for n in self.in_names
        })


_RUNNERS = {}


def _get_runner(cfg):
    key = (cfg["N"], cfg["E"], cfg["T_pad"], cfg["wv_wire"])
    r = _RUNNERS.get(key)
    if r is None:
        r = _Runner(cfg)
        _RUNNERS[key] = r
    return r


# ----------------------------------------------------------------------------
# entry point
# ----------------------------------------------------------------------------

def _in_maps(cfg, per_core):
    return [dict(blob=per_core["big"][c], xq=per_core["xq"][c])
            for c in range(N_CORES)]


def _global_in(cfg, per_core):
    # contiguous [8, r, c] -> [8*r, c] reshapes: zero-copy views
    big = per_core["big"]
    xq = per_core["xq"]
    return dict(blob=big.reshape(-1, big.shape[-1]),
                xq=xq.reshape(-1, xq.shape[-1]))


def _assemble(cfg, asm, core_outs):
    stacked = np.stack([co["out"] for co in core_outs])      # [8, 2, SHARD]
    out = stacked[asm["node_core"], :, asm["node_pos"]]       # [N, 2]
    return np.ascontiguousarray(out.astype(np.float32))


def kernel(**inputs):
    cfg, per_core, asm = _prep(inputs)
    runner = _get_runner(cfg)
    res = runner.run_global(_global_in(cfg, per_core))
    return _assemble(cfg, asm, res)
